# revision 17
# baseline (speedup 1.0000x reference)
"""Trainium2 Bass kernel for nn_CnnMulti2GruUser.

Model: token embedding gather -> per-sentence multi-ngram CNN (k=2..5,
128 filters each, relu, max-over-time, concat, proj to 256) -> sentence
BiGRU over 30 sentences per doc (batch=docs) -> doc vector = fwd+bwd last
hidden -> concat normalized user feats -> r_stars head; doc-sequence BiGRU
over the 64 docs (batch=1) -> p_stars head.

Sharding: data-parallel over docs (8 docs/core on 8 cores) for the
embedding+CNN+sentence-GRU; AllGather of [256+20, 64] doc vectors; the tiny
doc-sequence GRU + heads run replicated on every core (identical results,
core 0's output is read back).

All heavy matmuls run in bf16 (fp32 matmul is ~4x slower on PE), with fp32
PSUM accumulation and fp32 gate math in the GRUs.
"""

import sys
import types

if "/opt/trn_rl_repo" not in sys.path:
    sys.path.insert(0, "/opt/trn_rl_repo")

import numpy as np
import ml_dtypes

BF16 = ml_dtypes.bfloat16

# ---------------------------------------------------------------- ntff hook
# The agent image's antenv lacks axon_hooks; recreate it so trace=True works.
def _install_ntff_hook():
    if "antenv.axon_hooks" in sys.modules:
        return
    mod = types.ModuleType("antenv.axon_hooks")
    _hook = [None]
    mod.set_axon_ntff_profile_hook = lambda h: _hook.__setitem__(0, h)
    mod.get_axon_ntff_profile_hook = lambda: _hook[0]
    sys.modules["antenv.axon_hooks"] = mod
    try:
        import antenv

        antenv.axon_hooks = mod
        import trn_agent_boot.trn_boot as tb

        mod.set_axon_ntff_profile_hook(
            tb._ntff_profile_via_ctypes("/opt/axon/libaxon_pjrt.so")
        )
    except Exception:
        pass


_install_ntff_hook()

import concourse.bass as bass
import concourse.mybir as mybir
import concourse.tile as tile
from concourse.tile import TileContext
from concourse.vector_clock import ScopedClock
from concourse.bass_utils import run_bass_kernel_spmd

# ------------------------------------------------------- drain-wait split
# walrus trn2 codegen rejects Drain instructions with >2 sem waits; split the
# Tile kernel-tail drain's waits across single-wait NOPs on the same engine.
def _patched_drain_and_barrier(self, tick_clock, wait_clock):
    nc = self.nc
    drain_inst = nc.sync.drain()
    wait_clock.add_sem_waits(
        drain_inst.ins, ScopedClock({None: tick_clock.global_clock})
    )
    si = drain_inst.ins.sync_info
    if si is not None and len(si.on_wait) > 1:
        waits = list(si.on_wait)
        drain_inst.ins.sync_info = mybir.SyncInfo(
            on_wait=waits[:1], on_update=list(si.on_update)
        )
        for w in waits[1:]:
            ni = nc.sync.nop()
            ni.ins.sync_info = mybir.SyncInfo(on_wait=[w], on_update=[])
    nc.all_engine_barrier()
    popped = nc._tile_sem_poison_stack.pop()
    assert popped is self._sem_poison
    nc.clear_and_free_semaphores(list(self.sems.allocated().values()))
    nc.all_engine_barrier()


TileContext._drain_and_barrier = _patched_drain_and_barrier

# Same walrus limit applies to every instruction: at most 2 sem waits. Wrap
# Tile's commit step to front-run excess waits on same-engine NOPs.
_orig_commit = TileContext._commit_instruction


def _max_waits(inst):
    return 1


def _commit_split(self, inst, lazy_reg_writes=True):
    _MAXW = _max_waits(inst)
    si = getattr(inst, "sync_info", None)
    if si is not None and len(si.on_wait) > _MAXW:
        waits = list(si.on_wait)
        excess, keep = waits[:-_MAXW], waits[-_MAXW:]
        for i in range(0, len(excess), 1):
            nop = mybir.InstNoOp(
                name=f"{inst.name}-w{i}",
                sync_info=mybir.SyncInfo(on_wait=excess[i:i + 1],
                                         on_update=[]),
                bass_nofuse=True,
                engine=inst.engine,
            )
            _orig_commit(self, nop, lazy_reg_writes)
        inst.sync_info = mybir.SyncInfo(on_wait=keep,
                                        on_update=list(si.on_update))
    return _orig_commit(self, inst, lazy_reg_writes)


TileContext._commit_instruction = _commit_split

# ------------------------------------------------------------- dimensions
NGRAMS = (2, 3, 4, 5)
P, S, T = 64, 30, 128
V, E, H, NF, U = 50000, 200, 256, 128, 20
NCORES = 8
DPC = P // NCORES          # docs per core
NSENT = DPC * S            # sentences per core
BLK = 4                    # sentences per conv block
NBLK = NSENT // BLK
E0, E1 = 128, E - 128      # embedding-dim chunks (contraction on PE)
PAIRS = [(k, j) for k in NGRAMS for j in range(k)]   # 14 (ngram, shift)
NPAIR = len(PAIRS)
G3 = 3 * H                 # 768 gate units
GMC = G3 // 128            # 6 gate-unit chunks
HMC = H // 128             # 2 hidden chunks
PB = H + U                 # 276
SELU_L = 1.0507009873554805
SELU_A = 1.6732632423543772

F32 = mybir.dt.float32
BF = mybir.dt.bfloat16
I32 = mybir.dt.int32
AX = mybir.AxisListType
ALU = mybir.AluOpType
ACT = mybir.ActivationFunctionType

_cache = {}


def _build_nc():
    nc = bass.Bass("TRN2", target_bir_lowering=False, debug=False,
                   num_devices=NCORES)

    # ---------------- dram I/O ----------------
    embed = nc.dram_tensor("embed", [V, E], BF, kind="ExternalInput")
    toks = nc.dram_tensor("toks", [128, NSENT], I32, kind="ExternalInput")
    convw_d = nc.dram_tensor("convw", [128, NPAIR * 2 * 128], BF,
                             kind="ExternalInput")
    convb_d = nc.dram_tensor("convb", [128, 4], F32, kind="ExternalInput")
    projw_d = nc.dram_tensor("projw", [128, 4 * 2 * 128], BF,
                             kind="ExternalInput")
    projb_d = nc.dram_tensor("projb", [128, 2], F32, kind="ExternalInput")
    ident_d = nc.dram_tensor("ident", [128, 128], BF, kind="ExternalInput")
    # sentence GRU (dir-major: fwd, bwd)
    swih_d = nc.dram_tensor("swih", [128, 2 * 2 * GMC * 128], BF,
                            kind="ExternalInput")
    swhh_d = nc.dram_tensor("swhh", [128, 2 * 2 * GMC * 128], BF,
                            kind="ExternalInput")
    sgxb_d = nc.dram_tensor("sgxb", [128, 2 * GMC], F32, kind="ExternalInput")
    # doc GRU
    rwih_d = nc.dram_tensor("rwih", [128, 2 * 3 * GMC * 128], BF,
                            kind="ExternalInput")
    rwhh_d = nc.dram_tensor("rwhh", [128, 2 * 2 * GMC * 128], BF,
                            kind="ExternalInput")
    rgxb_d = nc.dram_tensor("rgxb", [128, 2 * GMC], F32, kind="ExternalInput")
    # heads
    rfc1_d = nc.dram_tensor("rfc1", [128, 3 * 128], BF, kind="ExternalInput")
    rfcb1_d = nc.dram_tensor("rfcb1", [128, 1], F32, kind="ExternalInput")
    rfc2_d = nc.dram_tensor("rfc2", [128, 1], BF, kind="ExternalInput")
    pfc1_d = nc.dram_tensor("pfc1", [128, 2 * 128], BF, kind="ExternalInput")
    pfcb1_d = nc.dram_tensor("pfcb1", [128, 1], F32, kind="ExternalInput")
    pfc2_d = nc.dram_tensor("pfc2", [128, 1], BF, kind="ExternalInput")
    fcb2_d = nc.dram_tensor("fcb2", [1, 2], F32, kind="ExternalInput")
    ufbf_d = nc.dram_tensor("ufbf", [U, P], BF, kind="ExternalInput")

    out_d = nc.dram_tensor("out", [1, 1 + P], F32, kind="ExternalOutput")

    with TileContext(nc) as tc:
        with (
            tc.tile_pool(name="wts", bufs=1) as wts,
            tc.tile_pool(name="persist", bufs=1) as persist,
            tc.tile_pool(name="gather", bufs=3) as gpool,
            tc.tile_pool(name="xt", bufs=3) as xtpool,
            tc.tile_pool(name="step", bufs=3) as spool,
            # PSUM budget (8 banks): ps0+ps1 bufs=2 -> 4, mm bufs=2 -> 2,
            # ghp bufs=2 -> 2.  All matmul tiles <=1 bank.
            tc.tile_pool(name="pc", bufs=6, space="PSUM") as pc,
            tc.tile_pool(name="pg", bufs=2, space="PSUM") as pg,
            tc.tile_pool(name="dram", bufs=1, space="DRAM") as dram,
        ):
            # ------------- load weights/constants to SBUF -------------
            def wtile(shape, dt, src, tag):
                t = wts.tile(shape, dt, tag=tag, name=tag)
                nc.sync.dma_start(t[:], src)
                return t

            convw = wtile([128, NPAIR * 2 * 128], BF, convw_d[:], "convw")
            convb = wtile([128, 4], F32, convb_d[:], "convb")
            projw = wtile([128, 4 * 2 * 128], BF, projw_d[:], "projw")
            projb = wtile([128, 2], F32, projb_d[:], "projb")
            ident = wtile([128, 128], BF, ident_d[:], "ident")
            swih = wtile([128, 2 * 2 * GMC * 128], BF, swih_d[:], "swih")
            swhh = wtile([128, 2 * 2 * GMC * 128], BF, swhh_d[:], "swhh")
            sgxb = wtile([128, 2 * GMC], F32, sgxb_d[:], "sgxb")
            rwih = wtile([128, 2 * 3 * GMC * 128], BF, rwih_d[:], "rwih")
            rwhh = wtile([128, 2 * 2 * GMC * 128], BF, rwhh_d[:], "rwhh")
            rgxb = wtile([128, 2 * GMC], F32, rgxb_d[:], "rgxb")
            rfc1 = wtile([128, 3 * 128], BF, rfc1_d[:], "rfc1")
            rfcb1 = wtile([128, 1], F32, rfcb1_d[:], "rfcb1")
            rfc2 = wtile([128, 1], BF, rfc2_d[:], "rfc2")
            pfc1 = wtile([128, 2 * 128], BF, pfc1_d[:], "pfc1")
            pfcb1 = wtile([128, 1], F32, pfcb1_d[:], "pfcb1")
            pfc2 = wtile([128, 1], BF, pfc2_d[:], "pfc2")
            fcb2 = wtile([1, 2], F32, fcb2_d[:], "fcb2")
            ufbf = wtile([U, P], BF, ufbf_d[:], "ufbf")
            toksb = wtile([128, NSENT], I32, toks[:], "toksb")

            def sw(dir_, kc, mc):   # sentence-GRU Wih^T block [128,128]
                return swih[:, ((dir_ * 2 + kc) * GMC + mc) * 128:
                            ((dir_ * 2 + kc) * GMC + mc) * 128 + 128]

            def sh(dir_, kc, mc):
                return swhh[:, ((dir_ * 2 + kc) * GMC + mc) * 128:
                            ((dir_ * 2 + kc) * GMC + mc) * 128 + 128]

            def rw(dir_, kc, mc):
                return rwih[:, ((dir_ * 3 + kc) * GMC + mc) * 128:
                            ((dir_ * 3 + kc) * GMC + mc) * 128 + 128]

            def rh(dir_, kc, mc):
                return rwhh[:, ((dir_ * 2 + kc) * GMC + mc) * 128:
                            ((dir_ * 2 + kc) * GMC + mc) * 128 + 128]

            # persistent activations
            pooled = [persist.tile([128, NSENT], BF, tag=f"pooled{i}",
                                   name=f"pooled{i}") for i in range(4)]
            svT = persist.tile([128, 2 * NSENT], BF, tag="svT")
            gxall = persist.tile([128, 2 * GMC * NSENT], F32, tag="gxall")
            hall32 = persist.tile([128, 2 * 2 * DPC], F32, tag="hall32")
            hallbf = persist.tile([128, 2 * 2 * DPC], BF, tag="hallbf")
            docv = persist.tile([128, 2 * DPC], F32, tag="docv")
            pbT = [persist.tile([128, P], F32, tag=f"pbT{c}", name=f"pbT{c}")
                   for c in range(2)]
            pbbf = [persist.tile([128, P], BF, tag=f"pbbf{c}", name=f"pbbf{c}")
                    for c in range(2)]
            gxrall = persist.tile([128, 2 * GMC * P], F32, tag="gxrall")
            hrall32 = persist.tile([128, 4], F32, tag="hrall32")
            hrallbf = persist.tile([128, 4], BF, tag="hrallbf")
            out_sb = persist.tile([1, 1 + P], F32, tag="out_sb")

            # ================= stage A: CNN encoder =================
            for b in range(NBLK):
                # 256-col slot per sentence: cols 200..256 are padding so the
                # E1 transpose below can read a full 128-wide window
                gbuf = gpool.tile([128, BLK * 256], BF, tag="gbuf")
                for s in range(BLK):
                    nc.gpsimd.indirect_dma_start(
                        out=gbuf[:, s * 256: s * 256 + E],
                        out_offset=None,
                        in_=embed[:],
                        in_offset=bass.IndirectOffsetOnAxis(
                            ap=toksb[:, b * BLK + s: b * BLK + s + 1], axis=0
                        ),
                    )
                # transpose to [E, tok] layout on the DMA engines (xbar
                # transpose, bf16) -- keeps the PE free for conv matmuls
                xt0 = xtpool.tile([128, BLK * 128], BF, tag="xt0")
                xt1 = xtpool.tile([128, BLK * 128], BF, tag="xt1")
                for s in range(BLK):
                    nc.sync.dma_start_transpose(
                        xt0[:, s * 128:(s + 1) * 128],
                        gbuf[:, s * 256: s * 256 + 128])
                    nc.sync.dma_start_transpose(
                        xt1[:, s * 128:(s + 1) * 128],
                        gbuf[:, s * 256 + 128: s * 256 + 256])

                xts = (xt0, xt1)
                kdim = (E0, E1)
                for ki, k in enumerate(NGRAMS):
                    L = T - k + 1
                    cps = pc.tile([128, BLK * L], F32, tag="mm")
                    n_mm = 2 * k
                    mi = 0
                    for j in range(k):
                        for c in range(2):
                            pi = PAIRS.index((k, j))
                            lhsT = convw[:kdim[c],
                                         (pi * 2 + c) * 128:(pi * 2 + c) * 128 + 128]
                            rhs = (xts[c][:kdim[c], :]
                                   .rearrange("p (s t) -> p s t", s=BLK)
                                   [:, :, j:j + L])
                            nc.tensor.matmul(
                                out=cps[:].rearrange("p (s t) -> p s t", s=BLK),
                                lhsT=lhsT, rhs=rhs,
                                start=(mi == 0), stop=(mi == n_mm - 1))
                            mi += 1
                    # max over time commutes with relu and the per-filter
                    # bias: pool first (cheap epilogue), relu+bias on [128,4]
                    poolf = spool.tile([128, BLK], F32, tag="poolf")
                    nc.vector.tensor_reduce(
                        out=poolf[:],
                        in_=cps[:].rearrange("p (s t) -> p s t", s=BLK),
                        axis=AX.X, op=ALU.max)
                    nc.scalar.activation(pooled[ki][:, b * BLK:(b + 1) * BLK],
                                         poolf[:], ACT.Relu,
                                         bias=convb[:, ki:ki + 1])

            # projection: svT[h] = proj_w.T @ pooled  (+bias)
            for h in range(2):
                pps = pc.tile([128, NSENT], F32, tag="mm")
                for c in range(4):
                    nc.tensor.matmul(
                        out=pps[:],
                        lhsT=projw[:, (c * 2 + h) * 128:(c * 2 + h) * 128 + 128],
                        rhs=pooled[c][:],
                        start=(c == 0), stop=(c == 3))
                nc.scalar.activation(svT[:, h * NSENT:(h + 1) * NSENT], pps[:],
                                     ACT.Identity, bias=projb[:, h:h + 1])

            # ================= stage B: sentence BiGRU =================
            # gx[d][:, mc*NSENT + s] = (Wih_d^T @ sv)[gate chunk mc, sentence s]
            for d in range(2):
                for mc in range(GMC):
                    gps = pc.tile([128, NSENT], F32, tag="mm")
                    for kc in range(2):
                        nc.tensor.matmul(
                            out=gps[:], lhsT=sw(d, kc, mc),
                            rhs=svT[:, kc * NSENT:(kc + 1) * NSENT],
                            start=(kc == 0), stop=(kc == 1))
                    nc.scalar.activation(
                        gxall[:, (d * GMC + mc) * NSENT:
                              (d * GMC + mc + 1) * NSENT], gps[:],
                        ACT.Identity, bias=sgxb[:, d * GMC + mc: d * GMC + mc + 1])
            nc.vector.memset(hall32[:], 0.0)
            nc.vector.memset(hallbf[:], 0.0)

            # both directions processed per step in the same instructions;
            # custom APs pair dir-0 column t with dir-1 column S-1-t.
            gxt = gxall[:].ap[0]
            for t in range(S):
                ghp = pg.tile([128, 2 * GMC * DPC], F32, tag="ghp")
                for d in range(2):
                    for mc in range(GMC):
                        for kc in range(2):
                            nc.tensor.matmul(
                                out=ghp[:, (d * GMC + mc) * DPC:
                                        (d * GMC + mc + 1) * DPC],
                                lhsT=sh(d, kc, mc),
                                rhs=hallbf[:, (d * 2 + kc) * DPC:
                                           (d * 2 + kc + 1) * DPC],
                                start=(kc == 0), stop=(kc == 1))
                dstep = GMC * NSENT + (S - 1 - 2 * t) * 1

                def gxsl(g0, ng):
                    return bass.AP(gxall[:].tensor, gxall[:].offset
                                   + g0 * NSENT + t,
                                   [gxt, [dstep, 2], [NSENT, ng], [S, DPC]])

                def ghsl(g0, ng):
                    return (ghp[:].rearrange("p (d g s) -> p d g s", d=2, g=GMC)
                            [:, :, g0:g0 + ng, :])
                rz = spool.tile([128, 2 * 4 * DPC], F32, tag="rz")
                nc.vector.tensor_add(
                    rz[:].rearrange("p (d g s) -> p d g s", d=2, g=4),
                    gxsl(0, 4), ghsl(0, 4))
                nc.scalar.activation(rz[:], rz[:], ACT.Sigmoid)
                npre = spool.tile([128, 2 * 2 * DPC], F32, tag="npre")
                nc.vector.tensor_mul(
                    npre[:].rearrange("p (d g s) -> p d g s", d=2, g=2),
                    rz[:].rearrange("p (d g s) -> p d g s", d=2, g=4)[:, :, 0:2, :],
                    ghsl(4, 2))
                nc.vector.tensor_add(
                    npre[:].rearrange("p (d g s) -> p d g s", d=2, g=2),
                    npre[:].rearrange("p (d g s) -> p d g s", d=2, g=2),
                    gxsl(4, 2))
                nc.scalar.activation(npre[:], npre[:], ACT.Tanh)
                hmn = spool.tile([128, 2 * 2 * DPC], F32, tag="hmn")
                nc.vector.tensor_sub(hmn[:], hall32[:], npre[:])
                nc.vector.tensor_mul(
                    hmn[:].rearrange("p (d g s) -> p d g s", d=2, g=2),
                    rz[:].rearrange("p (d g s) -> p d g s", d=2, g=4)[:, :, 2:4, :],
                    hmn[:].rearrange("p (d g s) -> p d g s", d=2, g=2))
                nc.vector.tensor_add(hall32[:], npre[:], hmn[:])
                nc.vector.tensor_add(hallbf[:], npre[:], hmn[:])

            nc.vector.tensor_add(docv[:], hall32[:, :2 * DPC],
                                 hall32[:, 2 * DPC:4 * DPC])

            # ================= AllGather doc vectors =================
            bounce = dram.tile([2, 128, DPC], F32, tag="bounce")
            gath = dram.tile([NCORES, 2, 128, DPC], F32, tag="gath")
            for c in range(2):
                nc.sync.dma_start(bounce[c], docv[:, c * DPC:(c + 1) * DPC])
            nc.gpsimd.collective_compute(
                "AllGather", ALU.bypass,
                replica_groups=[list(range(NCORES))],
                ins=[bounce.opt()], outs=[gath.opt()])
            for c in range(2):
                nc.sync.dma_start(
                    pbT[c][:],
                    gath[:, c, :, :].rearrange("a p b -> p a b"))
                nc.vector.tensor_copy(pbbf[c][:], pbT[c][:])

            # ================= r_stars head =================
            rps = pg.tile([128, P], F32, tag="ghp")
            for c in range(2):
                nc.tensor.matmul(out=rps[:], lhsT=rfc1[:, c * 128:(c + 1) * 128],
                                 rhs=pbbf[c][:], start=(c == 0), stop=False)
            nc.tensor.matmul(out=rps[:], lhsT=rfc1[:U, 2 * 128:3 * 128],
                             rhs=ufbf[:], start=False, stop=True)
            selu_in = spool.tile([128, P], F32, tag="selu_in")
            nc.scalar.activation(selu_in[:], rps[:], ACT.Identity,
                                 bias=rfcb1[:])

            def selu_apply(dst_bf, x, cols):
                # dst = SELU_L*relu(x) + SELU_L*SELU_A*(exp(min(x,0))-1)
                pos = spool.tile([128, P], F32, tag="selu_pos")
                neg = spool.tile([128, P], F32, tag="selu_neg")
                nc.scalar.activation(pos[:, :cols], x, ACT.Relu, scale=SELU_L)
                nc.vector.tensor_scalar_min(neg[:, :cols], x, 0.0)
                nc.scalar.activation(neg[:, :cols], neg[:, :cols], ACT.Exp)
                nc.vector.tensor_scalar(
                    out=neg[:, :cols], in0=neg[:, :cols],
                    scalar1=SELU_L * SELU_A, scalar2=-SELU_L * SELU_A,
                    op0=ALU.mult, op1=ALU.add)
                nc.vector.tensor_add(neg[:, :cols], neg[:, :cols], pos[:, :cols])
                nc.vector.tensor_copy(dst_bf, neg[:, :cols])

            selu_bf = spool.tile([128, P], BF, tag="selu_bf")
            selu_apply(selu_bf[:], selu_in[:], P)
            rsp = pg.tile([1, P], F32, tag="ghp")
            nc.tensor.matmul(out=rsp[:], lhsT=rfc2[:], rhs=selu_bf[:],
                             start=True, stop=True)
            nc.scalar.activation(out_sb[:, 1:1 + P], rsp[:], ACT.Identity,
                                 bias=fcb2[:, 0:1])

            # ================= doc-sequence BiGRU =================
            for d in range(2):
                for mc in range(GMC):
                    gps = pc.tile([128, P], F32, tag="mm")
                    nc.tensor.matmul(out=gps[:], lhsT=rw(d, 0, mc),
                                     rhs=pbbf[0][:], start=True, stop=False)
                    nc.tensor.matmul(out=gps[:], lhsT=rw(d, 1, mc),
                                     rhs=pbbf[1][:], start=False, stop=False)
                    nc.tensor.matmul(out=gps[:], lhsT=rw(d, 2, mc)[:U, :],
                                     rhs=ufbf[:], start=False, stop=True)
                    nc.scalar.activation(
                        gxrall[:, (d * GMC + mc) * P:(d * GMC + mc + 1) * P],
                        gps[:], ACT.Identity,
                        bias=rgxb[:, d * GMC + mc: d * GMC + mc + 1])
            nc.vector.memset(hrall32[:], 0.0)
            nc.vector.memset(hrallbf[:], 0.0)

            gxrt = gxrall[:].ap[0]
            for t in range(P):
                ghp = pg.tile([128, 2 * GMC], F32, tag="ghp")
                for d in range(2):
                    for mc in range(GMC):
                        for kc in range(2):
                            nc.tensor.matmul(
                                out=ghp[:, d * GMC + mc: d * GMC + mc + 1],
                                lhsT=rh(d, kc, mc),
                                rhs=hrallbf[:, d * 2 + kc: d * 2 + kc + 1],
                                start=(kc == 0), stop=(kc == 1))
                dstep = GMC * P + (P - 1 - 2 * t) * 1

                def gxrsl(g0, ng):
                    return bass.AP(gxrall[:].tensor, gxrall[:].offset
                                   + g0 * P + t,
                                   [gxrt, [dstep, 2], [P, ng]])

                def ghrsl(g0, ng):
                    return (ghp[:].rearrange("p (d g) -> p d g", d=2)
                            [:, :, g0:g0 + ng])
                rz = spool.tile([128, 8], F32, tag="rzr")
                nc.vector.tensor_add(
                    rz[:].rearrange("p (d g) -> p d g", d=2),
                    gxrsl(0, 4), ghrsl(0, 4))
                nc.scalar.activation(rz[:], rz[:], ACT.Sigmoid)
                npre = spool.tile([128, 4], F32, tag="nprer")
                nc.vector.tensor_mul(
                    npre[:].rearrange("p (d g) -> p d g", d=2),
                    rz[:].rearrange("p (d g) -> p d g", d=2)[:, :, 0:2],
                    ghrsl(4, 2))
                nc.vector.tensor_add(
                    npre[:].rearrange("p (d g) -> p d g", d=2),
                    npre[:].rearrange("p (d g) -> p d g", d=2),
                    gxrsl(4, 2))
                nc.scalar.activation(npre[:], npre[:], ACT.Tanh)
                hmn = spool.tile([128, 4], F32, tag="hmnr")
                nc.vector.tensor_sub(hmn[:], hrall32[:], npre[:])
                nc.vector.tensor_mul(
                    hmn[:].rearrange("p (d g) -> p d g", d=2),
                    rz[:].rearrange("p (d g) -> p d g", d=2)[:, :, 2:4],
                    hmn[:].rearrange("p (d g) -> p d g", d=2))
                nc.vector.tensor_add(hrall32[:], npre[:], hmn[:])
                nc.vector.tensor_add(hrallbf[:], npre[:], hmn[:])

            # ================= p_stars head =================
            hn = spool.tile([128, 2], F32, tag="hn")
            hnbf = spool.tile([128, 2], BF, tag="hnbf")
            nc.vector.tensor_add(hn[:], hrall32[:, 0:2], hrall32[:, 2:4])
            nc.vector.tensor_copy(hnbf[:], hn[:])
            pps1 = pg.tile([128, 1], F32, tag="ghp")
            for c in range(2):
                nc.tensor.matmul(out=pps1[:], lhsT=pfc1[:, c * 128:(c + 1) * 128],
                                 rhs=hnbf[:, c:c + 1],
                                 start=(c == 0), stop=(c == 1))
            psel_in = spool.tile([128, 1], F32, tag="psel_in")
            nc.scalar.activation(psel_in[:], pps1[:], ACT.Identity, bias=pfcb1[:])
            psel_bf = spool.tile([128, 1], BF, tag="psel_bf")
            selu_apply(psel_bf[:], psel_in[:], 1)
            pstar = pg.tile([1, 1], F32, tag="ghp")
            nc.tensor.matmul(out=pstar[:], lhsT=pfc2[:], rhs=psel_bf[:],
                             start=True, stop=True)
            nc.scalar.activation(out_sb[:, 0:1], pstar[:], ACT.Identity,
                                 bias=fcb2[:, 1:2])

            nc.sync.dma_start(out_d[:], out_sb[:])

    return nc


def _prep_inputs(inputs, user_feats, params):
    """Host-side packing of per-core input maps."""
    embed_bf = np.asarray(params["embed"], np.float32).astype(BF16)

    convw = np.zeros((128, NPAIR * 2 * 128), BF16)
    for pi, (k, j) in enumerate(PAIRS):
        Wt = np.asarray(params["conv_w"][str(k)], np.float32)[:, :, j].T  # [E,NF]
        convw[:, (pi * 2 + 0) * 128:(pi * 2 + 1) * 128] = Wt[:E0].astype(BF16)
        convw[:E1, (pi * 2 + 1) * 128:(pi * 2 + 2) * 128] = Wt[E0:].astype(BF16)
    convb = np.zeros((128, 4), np.float32)
    for ki, k in enumerate(NGRAMS):
        convb[:, ki] = np.asarray(params["conv_b"][str(k)], np.float32)

    pw = np.asarray(params["proj_w"], np.float32)  # [512, 256]
    projw = np.zeros((128, 4 * 2 * 128), BF16)
    for c in range(4):
        for h in range(2):
            projw[:, (c * 2 + h) * 128:(c * 2 + h + 1) * 128] = (
                pw[c * 128:(c + 1) * 128, h * 128:(h + 1) * 128].astype(BF16))
    projb = np.asarray(params["proj_b"], np.float32).reshape(2, 128).T.copy()
    ident = np.eye(128, dtype=np.float32).astype(BF16)

    def pack_gru(p, din, nkc):
        wih = np.zeros((128, 2 * nkc * GMC * 128), BF16)
        whh = np.zeros((128, 2 * 2 * GMC * 128), BF16)
        gxb = np.zeros((128, 2 * GMC), np.float32)
        for di, dname in enumerate(("fwd", "bwd")):
            g = p[dname]
            wihT = np.zeros((nkc * 128, G3), np.float32)
            wihT[:din] = np.asarray(g["wih"], np.float32).T
            whhT = np.asarray(g["whh"], np.float32).T  # [256, 768]
            for kc in range(nkc):
                for mc in range(GMC):
                    wih[:, ((di * nkc + kc) * GMC + mc) * 128:
                        ((di * nkc + kc) * GMC + mc + 1) * 128] = (
                        wihT[kc * 128:(kc + 1) * 128,
                             mc * 128:(mc + 1) * 128].astype(BF16))
            for kc in range(2):
                for mc in range(GMC):
                    whh[:, ((di * 2 + kc) * GMC + mc) * 128:
                        ((di * 2 + kc) * GMC + mc + 1) * 128] = (
                        whhT[kc * 128:(kc + 1) * 128,
                             mc * 128:(mc + 1) * 128].astype(BF16))
            bih = np.asarray(g["bih"], np.float32)
            bhh = np.asarray(g["bhh"], np.float32)
            assert np.all(bhh[2 * H:] == 0.0), "nonzero b_hn unsupported"
            btot = bih.copy()
            btot[:2 * H] += bhh[:2 * H]
            gxb[:, di * GMC:(di + 1) * GMC] = btot.reshape(GMC, 128).T
        return wih, whh, gxb

    swih, swhh, sgxb = pack_gru(params["s_gru"], H, 2)
    rwih, rwhh, rgxb = pack_gru(params["r_gru"], PB, 3)

    rf1 = np.asarray(params["rfc_w1"], np.float32)  # [276, 128]
    rfc1 = np.zeros((128, 3 * 128), BF16)
    rfc1[:, 0:128] = rf1[:128].astype(BF16)
    rfc1[:, 128:256] = rf1[128:256].astype(BF16)
    rfc1[:U, 256:384] = rf1[256:].astype(BF16)
    rfcb1 = np.asarray(params["rfc_b1"], np.float32).reshape(128, 1)
    rfc2 = np.asarray(params["rfc_w2"], np.float32).reshape(128, 1).astype(BF16)
    pf1 = np.asarray(params["pfc_w1"], np.float32)  # [256, 128]
    pfc1 = np.zeros((128, 2 * 128), BF16)
    pfc1[:, 0:128] = pf1[:128].astype(BF16)
    pfc1[:, 128:256] = pf1[128:].astype(BF16)
    pfcb1 = np.asarray(params["pfc_b1"], np.float32).reshape(128, 1)
    pfc2 = np.asarray(params["pfc_w2"], np.float32).reshape(128, 1).astype(BF16)
    fcb2 = np.array([[np.asarray(params["rfc_b2"], np.float32).reshape(()),
                      np.asarray(params["pfc_b2"], np.float32).reshape(())]],
                    np.float32)

    uf = np.asarray(user_feats, np.float32)
    ufn = uf / np.maximum(np.linalg.norm(uf, axis=-1, keepdims=True), 1e-12)
    ufbf = ufn.T.astype(BF16).copy()  # [20, 64]

    toks_all = np.asarray(inputs, np.int32)  # [P, S, T]
    shared = dict(
        embed=embed_bf, convw=convw, convb=convb, projw=projw, projb=projb,
        ident=ident, swih=swih, swhh=swhh, sgxb=sgxb, rwih=rwih, rwhh=rwhh,
        rgxb=rgxb, rfc1=rfc1, rfcb1=rfcb1, rfc2=rfc2, pfc1=pfc1, pfcb1=pfcb1,
        pfc2=pfc2, fcb2=fcb2, ufbf=ufbf,
    )
    in_maps = []
    for c in range(NCORES):
        t = toks_all[c * DPC:(c + 1) * DPC]          # [8, 30, 128]
        toks = np.ascontiguousarray(
            t.reshape(NSENT, T).T.astype(np.int32))  # [128, 240]
        in_maps.append(dict(shared, toks=toks))
    return in_maps


def kernel(inputs, sent_lengths, sent_counts, user_feats, params):
    if "nc" not in _cache:
        _cache["nc"] = _build_nc()
    nc = _cache["nc"]
    in_maps = _prep_inputs(inputs, user_feats, params)
    res = run_bass_kernel_spmd(nc, in_maps, list(range(NCORES)))
    out = res.results[0]["out"]
    p_stars = np.float32(out[0, 0])
    r_stars = out[0, 1:1 + P].astype(np.float32)
    return np.asarray(p_stars, np.float32), r_stars


def run_traced(inputs, sent_lengths, sent_counts, user_feats, params):
    """Like kernel() but returns (outputs, BassKernelResults-with-trace)."""
    if "nc" not in _cache:
        _cache["nc"] = _build_nc()
    nc = _cache["nc"]
    in_maps = _prep_inputs(inputs, user_feats, params)
    res = run_bass_kernel_spmd(nc, in_maps, list(range(NCORES)), trace=True)
    out = res.results[0]["out"]
    p_stars = np.float32(out[0, 0])
    r_stars = out[0, 1:1 + P].astype(np.float32)
    return (np.asarray(p_stars, np.float32), r_stars), res


# revision 18
# speedup vs baseline: 1.8112x; 1.8112x over previous
"""Trainium2 Bass kernel for nn_CnnMulti2GruUser.

Model: token embedding gather -> per-sentence multi-ngram CNN (k=2..5,
128 filters each, relu, max-over-time, concat, proj to 256) -> sentence
BiGRU over 30 sentences per doc (batch=docs) -> doc vector = fwd+bwd last
hidden -> concat normalized user feats -> r_stars head; doc-sequence BiGRU
over the 64 docs (batch=1) -> p_stars head.

Sharding: data-parallel over docs (8 docs/core on 8 cores) for the
embedding+CNN+sentence-GRU; AllGather of [256+20, 64] doc vectors; the tiny
doc-sequence GRU + heads run replicated on every core (identical results,
core 0's output is read back).

All heavy matmuls run in bf16 (fp32 matmul is ~4x slower on PE), with fp32
PSUM accumulation and fp32 gate math in the GRUs.
"""

import sys
import types

if "/opt/trn_rl_repo" not in sys.path:
    sys.path.insert(0, "/opt/trn_rl_repo")

import numpy as np
import ml_dtypes

BF16 = ml_dtypes.bfloat16

# ---------------------------------------------------------------- ntff hook
# The agent image's antenv lacks axon_hooks; recreate it so trace=True works.
def _install_ntff_hook():
    if "antenv.axon_hooks" in sys.modules:
        return
    mod = types.ModuleType("antenv.axon_hooks")
    _hook = [None]
    mod.set_axon_ntff_profile_hook = lambda h: _hook.__setitem__(0, h)
    mod.get_axon_ntff_profile_hook = lambda: _hook[0]
    sys.modules["antenv.axon_hooks"] = mod
    try:
        import antenv

        antenv.axon_hooks = mod
        import trn_agent_boot.trn_boot as tb

        mod.set_axon_ntff_profile_hook(
            tb._ntff_profile_via_ctypes("/opt/axon/libaxon_pjrt.so")
        )
    except Exception:
        pass


_install_ntff_hook()

import concourse.bass as bass
import concourse.mybir as mybir
import concourse.tile as tile
from concourse.tile import TileContext
from concourse.vector_clock import ScopedClock
from concourse.bass_utils import run_bass_kernel_spmd

# ------------------------------------------------------- drain-wait split
# walrus trn2 codegen rejects Drain instructions with >2 sem waits; split the
# Tile kernel-tail drain's waits across single-wait NOPs on the same engine.
def _patched_drain_and_barrier(self, tick_clock, wait_clock):
    nc = self.nc
    drain_inst = nc.sync.drain()
    wait_clock.add_sem_waits(
        drain_inst.ins, ScopedClock({None: tick_clock.global_clock})
    )
    si = drain_inst.ins.sync_info
    if si is not None and len(si.on_wait) > 1:
        waits = list(si.on_wait)
        drain_inst.ins.sync_info = mybir.SyncInfo(
            on_wait=waits[:1], on_update=list(si.on_update)
        )
        for w in waits[1:]:
            ni = nc.sync.nop()
            ni.ins.sync_info = mybir.SyncInfo(on_wait=[w], on_update=[])
    nc.all_engine_barrier()
    popped = nc._tile_sem_poison_stack.pop()
    assert popped is self._sem_poison
    nc.clear_and_free_semaphores(list(self.sems.allocated().values()))
    nc.all_engine_barrier()


TileContext._drain_and_barrier = _patched_drain_and_barrier

# Same walrus limit applies to every instruction: at most 2 sem waits. Wrap
# Tile's commit step to front-run excess waits on same-engine NOPs.
_orig_commit = TileContext._commit_instruction


def _max_waits(inst):
    return 1


def _commit_split(self, inst, lazy_reg_writes=True):
    _MAXW = _max_waits(inst)
    si = getattr(inst, "sync_info", None)
    if si is not None and len(si.on_wait) > _MAXW:
        waits = list(si.on_wait)
        excess, keep = waits[:-_MAXW], waits[-_MAXW:]
        for i in range(0, len(excess), 1):
            nop = mybir.InstNoOp(
                name=f"{inst.name}-w{i}",
                sync_info=mybir.SyncInfo(on_wait=excess[i:i + 1],
                                         on_update=[]),
                bass_nofuse=True,
                engine=inst.engine,
            )
            _orig_commit(self, nop, lazy_reg_writes)
        inst.sync_info = mybir.SyncInfo(on_wait=keep,
                                        on_update=list(si.on_update))
    return _orig_commit(self, inst, lazy_reg_writes)


TileContext._commit_instruction = _commit_split

# ------------------------------------------------------------- dimensions
NGRAMS = (2, 3, 4, 5)
P, S, T = 64, 30, 128
V, E, H, NF, U = 50000, 200, 256, 128, 20
NCORES = 8
DPC = P // NCORES          # docs per core
NSENT = DPC * S            # sentences per core
BLK = 4                    # sentences per conv block
NBLK = NSENT // BLK
E0, E1 = 128, E - 128      # embedding-dim chunks (contraction on PE)
PAIRS = [(k, j) for k in NGRAMS for j in range(k)]   # 14 (ngram, shift)
NPAIR = len(PAIRS)
G3 = 3 * H                 # 768 gate units
GMC = G3 // 128            # 6 gate-unit chunks
HMC = H // 128             # 2 hidden chunks
PB = H + U                 # 276
SELU_L = 1.0507009873554805
SELU_A = 1.6732632423543772

F32 = mybir.dt.float32
BF = mybir.dt.bfloat16
I32 = mybir.dt.int32
AX = mybir.AxisListType
ALU = mybir.AluOpType
ACT = mybir.ActivationFunctionType

_cache = {}


def _build_nc():
    nc = bass.Bass("TRN2", target_bir_lowering=False, debug=False,
                   num_devices=NCORES)

    # ---------------- dram I/O ----------------
    embed = nc.dram_tensor("embed", [V, E], BF, kind="ExternalInput")
    toks = nc.dram_tensor("toks", [128, NSENT], I32, kind="ExternalInput")
    convw_d = nc.dram_tensor("convw", [128, NPAIR * 2 * 128], BF,
                             kind="ExternalInput")
    convb_d = nc.dram_tensor("convb", [128, 4], F32, kind="ExternalInput")
    projw_d = nc.dram_tensor("projw", [128, 4 * 2 * 128], BF,
                             kind="ExternalInput")
    projb_d = nc.dram_tensor("projb", [128, 2], F32, kind="ExternalInput")
    ident_d = nc.dram_tensor("ident", [128, 128], BF, kind="ExternalInput")
    # sentence GRU (dir-major: fwd, bwd)
    swih_d = nc.dram_tensor("swih", [128, 2 * 2 * GMC * 128], BF,
                            kind="ExternalInput")
    swhh_d = nc.dram_tensor("swhh", [128, 2 * 2 * GMC * 128], BF,
                            kind="ExternalInput")
    sgxb_d = nc.dram_tensor("sgxb", [128, 2 * GMC], F32, kind="ExternalInput")
    # doc GRU
    rwih_d = nc.dram_tensor("rwih", [128, 2 * 3 * GMC * 128], BF,
                            kind="ExternalInput")
    rwhh_d = nc.dram_tensor("rwhh", [128, 2 * 2 * GMC * 128], BF,
                            kind="ExternalInput")
    rgxb_d = nc.dram_tensor("rgxb", [128, 2 * GMC], F32, kind="ExternalInput")
    # heads
    rfc1_d = nc.dram_tensor("rfc1", [128, 3 * 128], BF, kind="ExternalInput")
    rfcb1_d = nc.dram_tensor("rfcb1", [128, 1], F32, kind="ExternalInput")
    rfc2_d = nc.dram_tensor("rfc2", [128, 1], BF, kind="ExternalInput")
    pfc1_d = nc.dram_tensor("pfc1", [128, 2 * 128], BF, kind="ExternalInput")
    pfcb1_d = nc.dram_tensor("pfcb1", [128, 1], F32, kind="ExternalInput")
    pfc2_d = nc.dram_tensor("pfc2", [128, 1], BF, kind="ExternalInput")
    fcb2_d = nc.dram_tensor("fcb2", [1, 2], F32, kind="ExternalInput")
    ufbf_d = nc.dram_tensor("ufbf", [U, P], BF, kind="ExternalInput")

    out_d = nc.dram_tensor("out", [1, 1 + P], F32, kind="ExternalOutput")

    with TileContext(nc) as tc:
        with (
            tc.tile_pool(name="wts", bufs=1) as wts,
            tc.tile_pool(name="persist", bufs=1) as persist,
            tc.tile_pool(name="gather", bufs=3) as gpool,
            tc.tile_pool(name="xt", bufs=3) as xtpool,
            tc.tile_pool(name="step", bufs=3) as spool,
            # PSUM budget (8 banks): ps0+ps1 bufs=2 -> 4, mm bufs=2 -> 2,
            # ghp bufs=2 -> 2.  All matmul tiles <=1 bank.
            tc.tile_pool(name="pt", bufs=2, space="PSUM") as pt,
            tc.tile_pool(name="pc", bufs=2, space="PSUM") as pc,
            tc.tile_pool(name="pg", bufs=2, space="PSUM") as pg,
            tc.tile_pool(name="dram", bufs=1, space="DRAM") as dram,
        ):
            # ------------- load weights/constants to SBUF -------------
            def wtile(shape, dt, src, tag):
                t = wts.tile(shape, dt, tag=tag, name=tag)
                nc.sync.dma_start(t[:], src)
                return t

            convw = wtile([128, NPAIR * 2 * 128], BF, convw_d[:], "convw")
            convb = wtile([128, 4], F32, convb_d[:], "convb")
            projw = wtile([128, 4 * 2 * 128], BF, projw_d[:], "projw")
            projb = wtile([128, 2], F32, projb_d[:], "projb")
            ident = wtile([128, 128], BF, ident_d[:], "ident")
            swih = wtile([128, 2 * 2 * GMC * 128], BF, swih_d[:], "swih")
            swhh = wtile([128, 2 * 2 * GMC * 128], BF, swhh_d[:], "swhh")
            sgxb = wtile([128, 2 * GMC], F32, sgxb_d[:], "sgxb")
            rwih = wtile([128, 2 * 3 * GMC * 128], BF, rwih_d[:], "rwih")
            rwhh = wtile([128, 2 * 2 * GMC * 128], BF, rwhh_d[:], "rwhh")
            rgxb = wtile([128, 2 * GMC], F32, rgxb_d[:], "rgxb")
            rfc1 = wtile([128, 3 * 128], BF, rfc1_d[:], "rfc1")
            rfcb1 = wtile([128, 1], F32, rfcb1_d[:], "rfcb1")
            rfc2 = wtile([128, 1], BF, rfc2_d[:], "rfc2")
            pfc1 = wtile([128, 2 * 128], BF, pfc1_d[:], "pfc1")
            pfcb1 = wtile([128, 1], F32, pfcb1_d[:], "pfcb1")
            pfc2 = wtile([128, 1], BF, pfc2_d[:], "pfc2")
            fcb2 = wtile([1, 2], F32, fcb2_d[:], "fcb2")
            ufbf = wtile([U, P], BF, ufbf_d[:], "ufbf")
            toksb = wtile([128, NSENT], I32, toks[:], "toksb")

            def sw(dir_, kc, mc):   # sentence-GRU Wih^T block [128,128]
                return swih[:, ((dir_ * 2 + kc) * GMC + mc) * 128:
                            ((dir_ * 2 + kc) * GMC + mc) * 128 + 128]

            def sh(dir_, kc, mc):
                return swhh[:, ((dir_ * 2 + kc) * GMC + mc) * 128:
                            ((dir_ * 2 + kc) * GMC + mc) * 128 + 128]

            def rw(dir_, kc, mc):
                return rwih[:, ((dir_ * 3 + kc) * GMC + mc) * 128:
                            ((dir_ * 3 + kc) * GMC + mc) * 128 + 128]

            def rh(dir_, kc, mc):
                return rwhh[:, ((dir_ * 2 + kc) * GMC + mc) * 128:
                            ((dir_ * 2 + kc) * GMC + mc) * 128 + 128]

            # persistent activations
            pooled = [persist.tile([128, NSENT], BF, tag=f"pooled{i}",
                                   name=f"pooled{i}") for i in range(4)]
            svT = persist.tile([128, 2 * NSENT], BF, tag="svT")
            gxall = persist.tile([128, 2 * GMC * NSENT], F32, tag="gxall")
            hall32 = persist.tile([128, 2 * 2 * DPC], F32, tag="hall32")
            hallbf = persist.tile([128, 2 * 2 * DPC], BF, tag="hallbf")
            docv = persist.tile([128, 2 * DPC], F32, tag="docv")
            pbT = [persist.tile([128, P], F32, tag=f"pbT{c}", name=f"pbT{c}")
                   for c in range(2)]
            pbbf = [persist.tile([128, P], BF, tag=f"pbbf{c}", name=f"pbbf{c}")
                    for c in range(2)]
            gxrall = persist.tile([128, 2 * GMC * P], F32, tag="gxrall")
            hrall32 = persist.tile([128, 4], F32, tag="hrall32")
            hrallbf = persist.tile([128, 4], BF, tag="hrallbf")
            out_sb = persist.tile([1, 1 + P], F32, tag="out_sb")

            # ================= stage A: CNN encoder =================
            for b in range(NBLK):
                # 256-col slot per sentence: cols 200..256 are padding so the
                # E1 transpose below can read a full 128-wide window
                gbuf = gpool.tile([128, BLK * 256], BF, tag="gbuf")
                for s in range(BLK):
                    nc.gpsimd.indirect_dma_start(
                        out=gbuf[:, s * 256: s * 256 + E],
                        out_offset=None,
                        in_=embed[:],
                        in_offset=bass.IndirectOffsetOnAxis(
                            ap=toksb[:, b * BLK + s: b * BLK + s + 1], axis=0
                        ),
                    )
                # transpose to [E, tok] layout (PE transpose-mode)
                ps0 = pt.tile([128, BLK * 128], BF, tag="ps0")
                ps1 = pt.tile([128, BLK * 128], BF, tag="ps1")
                for s in range(BLK):
                    nc.tensor.transpose(
                        ps0[:, s * 128:(s + 1) * 128],
                        gbuf[:, s * 256: s * 256 + E0], ident[:])
                    nc.tensor.transpose(
                        ps1[:E1, s * 128:(s + 1) * 128],
                        gbuf[:, s * 256 + E0: s * 256 + E], ident[:])
                xt0 = xtpool.tile([128, BLK * 128], BF, tag="xt0")
                xt1 = xtpool.tile([128, BLK * 128], BF, tag="xt1")
                nc.scalar.copy(xt0[:], ps0[:])
                nc.vector.tensor_copy(xt1[:E1, :], ps1[:E1, :])

                xts = (xt0, xt1)
                kdim = (E0, E1)
                for ki, k in enumerate(NGRAMS):
                    L = T - k + 1
                    cps = pc.tile([128, BLK * L], F32, tag="mm")
                    n_mm = 2 * k
                    mi = 0
                    for j in range(k):
                        for c in range(2):
                            pi = PAIRS.index((k, j))
                            lhsT = convw[:kdim[c],
                                         (pi * 2 + c) * 128:(pi * 2 + c) * 128 + 128]
                            rhs = (xts[c][:kdim[c], :]
                                   .rearrange("p (s t) -> p s t", s=BLK)
                                   [:, :, j:j + L])
                            nc.tensor.matmul(
                                out=cps[:].rearrange("p (s t) -> p s t", s=BLK),
                                lhsT=lhsT, rhs=rhs,
                                start=(mi == 0), stop=(mi == n_mm - 1))
                            mi += 1
                    # max over time commutes with relu and the per-filter
                    # bias: pool first (cheap epilogue), relu+bias on [128,4]
                    poolf = spool.tile([128, BLK], F32, tag="poolf")
                    nc.vector.tensor_reduce(
                        out=poolf[:],
                        in_=cps[:].rearrange("p (s t) -> p s t", s=BLK),
                        axis=AX.X, op=ALU.max)
                    nc.scalar.activation(pooled[ki][:, b * BLK:(b + 1) * BLK],
                                         poolf[:], ACT.Relu,
                                         bias=convb[:, ki:ki + 1])

            # projection: svT[h] = proj_w.T @ pooled  (+bias)
            for h in range(2):
                pps = pc.tile([128, NSENT], F32, tag="mm")
                for c in range(4):
                    nc.tensor.matmul(
                        out=pps[:],
                        lhsT=projw[:, (c * 2 + h) * 128:(c * 2 + h) * 128 + 128],
                        rhs=pooled[c][:],
                        start=(c == 0), stop=(c == 3))
                nc.scalar.activation(svT[:, h * NSENT:(h + 1) * NSENT], pps[:],
                                     ACT.Identity, bias=projb[:, h:h + 1])

            # ================= stage B: sentence BiGRU =================
            # gx[d][:, mc*NSENT + s] = (Wih_d^T @ sv)[gate chunk mc, sentence s]
            for d in range(2):
                for mc in range(GMC):
                    gps = pc.tile([128, NSENT], F32, tag="mm")
                    for kc in range(2):
                        nc.tensor.matmul(
                            out=gps[:], lhsT=sw(d, kc, mc),
                            rhs=svT[:, kc * NSENT:(kc + 1) * NSENT],
                            start=(kc == 0), stop=(kc == 1))
                    nc.scalar.activation(
                        gxall[:, (d * GMC + mc) * NSENT:
                              (d * GMC + mc + 1) * NSENT], gps[:],
                        ACT.Identity, bias=sgxb[:, d * GMC + mc: d * GMC + mc + 1])
            nc.vector.memset(hall32[:], 0.0)
            nc.vector.memset(hallbf[:], 0.0)

            # both directions processed per step in the same instructions;
            # custom APs pair dir-0 column t with dir-1 column S-1-t.
            gxt = gxall[:].ap[0]
            for t in range(S):
                ghp = pg.tile([128, 2 * GMC * DPC], F32, tag="ghp")
                for d in range(2):
                    for mc in range(GMC):
                        for kc in range(2):
                            nc.tensor.matmul(
                                out=ghp[:, (d * GMC + mc) * DPC:
                                        (d * GMC + mc + 1) * DPC],
                                lhsT=sh(d, kc, mc),
                                rhs=hallbf[:, (d * 2 + kc) * DPC:
                                           (d * 2 + kc + 1) * DPC],
                                start=(kc == 0), stop=(kc == 1))
                dstep = GMC * NSENT + (S - 1 - 2 * t) * 1

                def gxsl(g0, ng):
                    return bass.AP(gxall[:].tensor, gxall[:].offset
                                   + g0 * NSENT + t,
                                   [gxt, [dstep, 2], [NSENT, ng], [S, DPC]])

                def ghsl(g0, ng):
                    return (ghp[:].rearrange("p (d g s) -> p d g s", d=2, g=GMC)
                            [:, :, g0:g0 + ng, :])
                rz = spool.tile([128, 2 * 4 * DPC], F32, tag="rz")
                nc.vector.tensor_add(
                    rz[:].rearrange("p (d g s) -> p d g s", d=2, g=4),
                    gxsl(0, 4), ghsl(0, 4))
                nc.scalar.activation(rz[:], rz[:], ACT.Sigmoid)
                npre = spool.tile([128, 2 * 2 * DPC], F32, tag="npre")
                nc.vector.tensor_mul(
                    npre[:].rearrange("p (d g s) -> p d g s", d=2, g=2),
                    rz[:].rearrange("p (d g s) -> p d g s", d=2, g=4)[:, :, 0:2, :],
                    ghsl(4, 2))
                nc.vector.tensor_add(
                    npre[:].rearrange("p (d g s) -> p d g s", d=2, g=2),
                    npre[:].rearrange("p (d g s) -> p d g s", d=2, g=2),
                    gxsl(4, 2))
                nc.scalar.activation(npre[:], npre[:], ACT.Tanh)
                hmn = spool.tile([128, 2 * 2 * DPC], F32, tag="hmn")
                nc.vector.tensor_sub(hmn[:], hall32[:], npre[:])
                nc.vector.tensor_mul(
                    hmn[:].rearrange("p (d g s) -> p d g s", d=2, g=2),
                    rz[:].rearrange("p (d g s) -> p d g s", d=2, g=4)[:, :, 2:4, :],
                    hmn[:].rearrange("p (d g s) -> p d g s", d=2, g=2))
                nc.vector.tensor_add(hall32[:], npre[:], hmn[:])
                nc.vector.tensor_add(hallbf[:], npre[:], hmn[:])

            nc.vector.tensor_add(docv[:], hall32[:, :2 * DPC],
                                 hall32[:, 2 * DPC:4 * DPC])

            # ================= AllGather doc vectors =================
            bounce = dram.tile([2, 128, DPC], F32, tag="bounce")
            gath = dram.tile([NCORES, 2, 128, DPC], F32, tag="gath")
            for c in range(2):
                nc.sync.dma_start(bounce[c], docv[:, c * DPC:(c + 1) * DPC])
            nc.gpsimd.collective_compute(
                "AllGather", ALU.bypass,
                replica_groups=[list(range(NCORES))],
                ins=[bounce.opt()], outs=[gath.opt()])
            for c in range(2):
                nc.sync.dma_start(
                    pbT[c][:],
                    gath[:, c, :, :].rearrange("a p b -> p a b"))
                nc.vector.tensor_copy(pbbf[c][:], pbT[c][:])

            # ================= r_stars head =================
            rps = pg.tile([128, P], F32, tag="ghp")
            for c in range(2):
                nc.tensor.matmul(out=rps[:], lhsT=rfc1[:, c * 128:(c + 1) * 128],
                                 rhs=pbbf[c][:], start=(c == 0), stop=False)
            nc.tensor.matmul(out=rps[:], lhsT=rfc1[:U, 2 * 128:3 * 128],
                             rhs=ufbf[:], start=False, stop=True)
            selu_in = spool.tile([128, P], F32, tag="selu_in")
            nc.scalar.activation(selu_in[:], rps[:], ACT.Identity,
                                 bias=rfcb1[:])

            def selu_apply(dst_bf, x, cols):
                # dst = SELU_L*relu(x) + SELU_L*SELU_A*(exp(min(x,0))-1)
                pos = spool.tile([128, P], F32, tag="selu_pos")
                neg = spool.tile([128, P], F32, tag="selu_neg")
                nc.scalar.activation(pos[:, :cols], x, ACT.Relu, scale=SELU_L)
                nc.vector.tensor_scalar_min(neg[:, :cols], x, 0.0)
                nc.scalar.activation(neg[:, :cols], neg[:, :cols], ACT.Exp)
                nc.vector.tensor_scalar(
                    out=neg[:, :cols], in0=neg[:, :cols],
                    scalar1=SELU_L * SELU_A, scalar2=-SELU_L * SELU_A,
                    op0=ALU.mult, op1=ALU.add)
                nc.vector.tensor_add(neg[:, :cols], neg[:, :cols], pos[:, :cols])
                nc.vector.tensor_copy(dst_bf, neg[:, :cols])

            selu_bf = spool.tile([128, P], BF, tag="selu_bf")
            selu_apply(selu_bf[:], selu_in[:], P)
            rsp = pg.tile([1, P], F32, tag="ghp")
            nc.tensor.matmul(out=rsp[:], lhsT=rfc2[:], rhs=selu_bf[:],
                             start=True, stop=True)
            nc.scalar.activation(out_sb[:, 1:1 + P], rsp[:], ACT.Identity,
                                 bias=fcb2[:, 0:1])

            # ================= doc-sequence BiGRU =================
            for d in range(2):
                for mc in range(GMC):
                    gps = pc.tile([128, P], F32, tag="mm")
                    nc.tensor.matmul(out=gps[:], lhsT=rw(d, 0, mc),
                                     rhs=pbbf[0][:], start=True, stop=False)
                    nc.tensor.matmul(out=gps[:], lhsT=rw(d, 1, mc),
                                     rhs=pbbf[1][:], start=False, stop=False)
                    nc.tensor.matmul(out=gps[:], lhsT=rw(d, 2, mc)[:U, :],
                                     rhs=ufbf[:], start=False, stop=True)
                    nc.scalar.activation(
                        gxrall[:, (d * GMC + mc) * P:(d * GMC + mc + 1) * P],
                        gps[:], ACT.Identity,
                        bias=rgxb[:, d * GMC + mc: d * GMC + mc + 1])
            nc.vector.memset(hrall32[:], 0.0)
            nc.vector.memset(hrallbf[:], 0.0)

            gxrt = gxrall[:].ap[0]
            for t in range(P):
                ghp = pg.tile([128, 2 * GMC], F32, tag="ghp")
                for d in range(2):
                    for mc in range(GMC):
                        for kc in range(2):
                            nc.tensor.matmul(
                                out=ghp[:, d * GMC + mc: d * GMC + mc + 1],
                                lhsT=rh(d, kc, mc),
                                rhs=hrallbf[:, d * 2 + kc: d * 2 + kc + 1],
                                start=(kc == 0), stop=(kc == 1))
                dstep = GMC * P + (P - 1 - 2 * t) * 1

                def gxrsl(g0, ng):
                    return bass.AP(gxrall[:].tensor, gxrall[:].offset
                                   + g0 * P + t,
                                   [gxrt, [dstep, 2], [P, ng]])

                def ghrsl(g0, ng):
                    return (ghp[:].rearrange("p (d g) -> p d g", d=2)
                            [:, :, g0:g0 + ng])
                rz = spool.tile([128, 8], F32, tag="rzr")
                nc.vector.tensor_add(
                    rz[:].rearrange("p (d g) -> p d g", d=2),
                    gxrsl(0, 4), ghrsl(0, 4))
                nc.scalar.activation(rz[:], rz[:], ACT.Sigmoid)
                npre = spool.tile([128, 4], F32, tag="nprer")
                nc.vector.tensor_mul(
                    npre[:].rearrange("p (d g) -> p d g", d=2),
                    rz[:].rearrange("p (d g) -> p d g", d=2)[:, :, 0:2],
                    ghrsl(4, 2))
                nc.vector.tensor_add(
                    npre[:].rearrange("p (d g) -> p d g", d=2),
                    npre[:].rearrange("p (d g) -> p d g", d=2),
                    gxrsl(4, 2))
                nc.scalar.activation(npre[:], npre[:], ACT.Tanh)
                hmn = spool.tile([128, 4], F32, tag="hmnr")
                nc.vector.tensor_sub(hmn[:], hrall32[:], npre[:])
                nc.vector.tensor_mul(
                    hmn[:].rearrange("p (d g) -> p d g", d=2),
                    rz[:].rearrange("p (d g) -> p d g", d=2)[:, :, 2:4],
                    hmn[:].rearrange("p (d g) -> p d g", d=2))
                nc.vector.tensor_add(hrall32[:], npre[:], hmn[:])
                nc.vector.tensor_add(hrallbf[:], npre[:], hmn[:])

            # ================= p_stars head =================
            hn = spool.tile([128, 2], F32, tag="hn")
            hnbf = spool.tile([128, 2], BF, tag="hnbf")
            nc.vector.tensor_add(hn[:], hrall32[:, 0:2], hrall32[:, 2:4])
            nc.vector.tensor_copy(hnbf[:], hn[:])
            pps1 = pg.tile([128, 1], F32, tag="ghp")
            for c in range(2):
                nc.tensor.matmul(out=pps1[:], lhsT=pfc1[:, c * 128:(c + 1) * 128],
                                 rhs=hnbf[:, c:c + 1],
                                 start=(c == 0), stop=(c == 1))
            psel_in = spool.tile([128, 1], F32, tag="psel_in")
            nc.scalar.activation(psel_in[:], pps1[:], ACT.Identity, bias=pfcb1[:])
            psel_bf = spool.tile([128, 1], BF, tag="psel_bf")
            selu_apply(psel_bf[:], psel_in[:], 1)
            pstar = pg.tile([1, 1], F32, tag="ghp")
            nc.tensor.matmul(out=pstar[:], lhsT=pfc2[:], rhs=psel_bf[:],
                             start=True, stop=True)
            nc.scalar.activation(out_sb[:, 0:1], pstar[:], ACT.Identity,
                                 bias=fcb2[:, 1:2])

            nc.sync.dma_start(out_d[:], out_sb[:])

    return nc


def _prep_inputs(inputs, user_feats, params):
    """Host-side packing of per-core input maps."""
    embed_bf = np.asarray(params["embed"], np.float32).astype(BF16)

    convw = np.zeros((128, NPAIR * 2 * 128), BF16)
    for pi, (k, j) in enumerate(PAIRS):
        Wt = np.asarray(params["conv_w"][str(k)], np.float32)[:, :, j].T  # [E,NF]
        convw[:, (pi * 2 + 0) * 128:(pi * 2 + 1) * 128] = Wt[:E0].astype(BF16)
        convw[:E1, (pi * 2 + 1) * 128:(pi * 2 + 2) * 128] = Wt[E0:].astype(BF16)
    convb = np.zeros((128, 4), np.float32)
    for ki, k in enumerate(NGRAMS):
        convb[:, ki] = np.asarray(params["conv_b"][str(k)], np.float32)

    pw = np.asarray(params["proj_w"], np.float32)  # [512, 256]
    projw = np.zeros((128, 4 * 2 * 128), BF16)
    for c in range(4):
        for h in range(2):
            projw[:, (c * 2 + h) * 128:(c * 2 + h + 1) * 128] = (
                pw[c * 128:(c + 1) * 128, h * 128:(h + 1) * 128].astype(BF16))
    projb = np.asarray(params["proj_b"], np.float32).reshape(2, 128).T.copy()
    ident = np.eye(128, dtype=np.float32).astype(BF16)

    def pack_gru(p, din, nkc):
        wih = np.zeros((128, 2 * nkc * GMC * 128), BF16)
        whh = np.zeros((128, 2 * 2 * GMC * 128), BF16)
        gxb = np.zeros((128, 2 * GMC), np.float32)
        for di, dname in enumerate(("fwd", "bwd")):
            g = p[dname]
            wihT = np.zeros((nkc * 128, G3), np.float32)
            wihT[:din] = np.asarray(g["wih"], np.float32).T
            whhT = np.asarray(g["whh"], np.float32).T  # [256, 768]
            for kc in range(nkc):
                for mc in range(GMC):
                    wih[:, ((di * nkc + kc) * GMC + mc) * 128:
                        ((di * nkc + kc) * GMC + mc + 1) * 128] = (
                        wihT[kc * 128:(kc + 1) * 128,
                             mc * 128:(mc + 1) * 128].astype(BF16))
            for kc in range(2):
                for mc in range(GMC):
                    whh[:, ((di * 2 + kc) * GMC + mc) * 128:
                        ((di * 2 + kc) * GMC + mc + 1) * 128] = (
                        whhT[kc * 128:(kc + 1) * 128,
                             mc * 128:(mc + 1) * 128].astype(BF16))
            bih = np.asarray(g["bih"], np.float32)
            bhh = np.asarray(g["bhh"], np.float32)
            assert np.all(bhh[2 * H:] == 0.0), "nonzero b_hn unsupported"
            btot = bih.copy()
            btot[:2 * H] += bhh[:2 * H]
            gxb[:, di * GMC:(di + 1) * GMC] = btot.reshape(GMC, 128).T
        return wih, whh, gxb

    swih, swhh, sgxb = pack_gru(params["s_gru"], H, 2)
    rwih, rwhh, rgxb = pack_gru(params["r_gru"], PB, 3)

    rf1 = np.asarray(params["rfc_w1"], np.float32)  # [276, 128]
    rfc1 = np.zeros((128, 3 * 128), BF16)
    rfc1[:, 0:128] = rf1[:128].astype(BF16)
    rfc1[:, 128:256] = rf1[128:256].astype(BF16)
    rfc1[:U, 256:384] = rf1[256:].astype(BF16)
    rfcb1 = np.asarray(params["rfc_b1"], np.float32).reshape(128, 1)
    rfc2 = np.asarray(params["rfc_w2"], np.float32).reshape(128, 1).astype(BF16)
    pf1 = np.asarray(params["pfc_w1"], np.float32)  # [256, 128]
    pfc1 = np.zeros((128, 2 * 128), BF16)
    pfc1[:, 0:128] = pf1[:128].astype(BF16)
    pfc1[:, 128:256] = pf1[128:].astype(BF16)
    pfcb1 = np.asarray(params["pfc_b1"], np.float32).reshape(128, 1)
    pfc2 = np.asarray(params["pfc_w2"], np.float32).reshape(128, 1).astype(BF16)
    fcb2 = np.array([[np.asarray(params["rfc_b2"], np.float32).reshape(()),
                      np.asarray(params["pfc_b2"], np.float32).reshape(())]],
                    np.float32)

    uf = np.asarray(user_feats, np.float32)
    ufn = uf / np.maximum(np.linalg.norm(uf, axis=-1, keepdims=True), 1e-12)
    ufbf = ufn.T.astype(BF16).copy()  # [20, 64]

    toks_all = np.asarray(inputs, np.int32)  # [P, S, T]
    shared = dict(
        embed=embed_bf, convw=convw, convb=convb, projw=projw, projb=projb,
        ident=ident, swih=swih, swhh=swhh, sgxb=sgxb, rwih=rwih, rwhh=rwhh,
        rgxb=rgxb, rfc1=rfc1, rfcb1=rfcb1, rfc2=rfc2, pfc1=pfc1, pfcb1=pfcb1,
        pfc2=pfc2, fcb2=fcb2, ufbf=ufbf,
    )
    in_maps = []
    for c in range(NCORES):
        t = toks_all[c * DPC:(c + 1) * DPC]          # [8, 30, 128]
        toks = np.ascontiguousarray(
            t.reshape(NSENT, T).T.astype(np.int32))  # [128, 240]
        in_maps.append(dict(shared, toks=toks))
    return in_maps


def kernel(inputs, sent_lengths, sent_counts, user_feats, params):
    if "nc" not in _cache:
        _cache["nc"] = _build_nc()
    nc = _cache["nc"]
    in_maps = _prep_inputs(inputs, user_feats, params)
    res = run_bass_kernel_spmd(nc, in_maps, list(range(NCORES)))
    out = res.results[0]["out"]
    p_stars = np.float32(out[0, 0])
    r_stars = out[0, 1:1 + P].astype(np.float32)
    return np.asarray(p_stars, np.float32), r_stars


def run_traced(inputs, sent_lengths, sent_counts, user_feats, params):
    """Like kernel() but returns (outputs, BassKernelResults-with-trace)."""
    if "nc" not in _cache:
        _cache["nc"] = _build_nc()
    nc = _cache["nc"]
    in_maps = _prep_inputs(inputs, user_feats, params)
    res = run_bass_kernel_spmd(nc, in_maps, list(range(NCORES)), trace=True)
    out = res.results[0]["out"]
    p_stars = np.float32(out[0, 0])
    r_stars = out[0, 1:1 + P].astype(np.float32)
    return (np.asarray(p_stars, np.float32), r_stars), res


# revision 19
# speedup vs baseline: 1.8389x; 1.0153x over previous
"""Trainium2 Bass kernel for nn_CnnMulti2GruUser.

Model: token embedding gather -> per-sentence multi-ngram CNN (k=2..5,
128 filters each, relu, max-over-time, concat, proj to 256) -> sentence
BiGRU over 30 sentences per doc (batch=docs) -> doc vector = fwd+bwd last
hidden -> concat normalized user feats -> r_stars head; doc-sequence BiGRU
over the 64 docs (batch=1) -> p_stars head.

Sharding: data-parallel over docs (8 docs/core on 8 cores) for the
embedding+CNN+sentence-GRU; AllGather of [256+20, 64] doc vectors; the tiny
doc-sequence GRU + heads run replicated on every core (identical results,
core 0's output is read back).

All heavy matmuls run in bf16 (fp32 matmul is ~4x slower on PE), with fp32
PSUM accumulation and fp32 gate math in the GRUs.
"""

import sys
import types

if "/opt/trn_rl_repo" not in sys.path:
    sys.path.insert(0, "/opt/trn_rl_repo")

import numpy as np
import ml_dtypes

BF16 = ml_dtypes.bfloat16

# ---------------------------------------------------------------- ntff hook
# The agent image's antenv lacks axon_hooks; recreate it so trace=True works.
def _install_ntff_hook():
    if "antenv.axon_hooks" in sys.modules:
        return
    mod = types.ModuleType("antenv.axon_hooks")
    _hook = [None]
    mod.set_axon_ntff_profile_hook = lambda h: _hook.__setitem__(0, h)
    mod.get_axon_ntff_profile_hook = lambda: _hook[0]
    sys.modules["antenv.axon_hooks"] = mod
    try:
        import antenv

        antenv.axon_hooks = mod
        import trn_agent_boot.trn_boot as tb

        mod.set_axon_ntff_profile_hook(
            tb._ntff_profile_via_ctypes("/opt/axon/libaxon_pjrt.so")
        )
    except Exception:
        pass


_install_ntff_hook()

import concourse.bass as bass
import concourse.mybir as mybir
import concourse.tile as tile
from concourse.tile import TileContext
from concourse.vector_clock import ScopedClock
from concourse.bass_utils import run_bass_kernel_spmd

# ------------------------------------------------------- drain-wait split
# walrus trn2 codegen rejects Drain instructions with >2 sem waits; split the
# Tile kernel-tail drain's waits across single-wait NOPs on the same engine.
def _patched_drain_and_barrier(self, tick_clock, wait_clock):
    nc = self.nc
    drain_inst = nc.sync.drain()
    wait_clock.add_sem_waits(
        drain_inst.ins, ScopedClock({None: tick_clock.global_clock})
    )
    si = drain_inst.ins.sync_info
    if si is not None and len(si.on_wait) > 1:
        waits = list(si.on_wait)
        drain_inst.ins.sync_info = mybir.SyncInfo(
            on_wait=waits[:1], on_update=list(si.on_update)
        )
        for w in waits[1:]:
            ni = nc.sync.nop()
            ni.ins.sync_info = mybir.SyncInfo(on_wait=[w], on_update=[])
    nc.all_engine_barrier()
    popped = nc._tile_sem_poison_stack.pop()
    assert popped is self._sem_poison
    nc.clear_and_free_semaphores(list(self.sems.allocated().values()))
    nc.all_engine_barrier()


TileContext._drain_and_barrier = _patched_drain_and_barrier

# Same walrus limit applies to every instruction: at most 2 sem waits. Wrap
# Tile's commit step to front-run excess waits on same-engine NOPs.
_orig_commit = TileContext._commit_instruction


def _max_waits(inst):
    return 1


def _commit_split(self, inst, lazy_reg_writes=True):
    _MAXW = _max_waits(inst)
    si = getattr(inst, "sync_info", None)
    if si is not None and len(si.on_wait) > _MAXW:
        waits = list(si.on_wait)
        excess, keep = waits[:-_MAXW], waits[-_MAXW:]
        for i in range(0, len(excess), 1):
            nop = mybir.InstNoOp(
                name=f"{inst.name}-w{i}",
                sync_info=mybir.SyncInfo(on_wait=excess[i:i + 1],
                                         on_update=[]),
                bass_nofuse=True,
                engine=inst.engine,
            )
            _orig_commit(self, nop, lazy_reg_writes)
        inst.sync_info = mybir.SyncInfo(on_wait=keep,
                                        on_update=list(si.on_update))
    return _orig_commit(self, inst, lazy_reg_writes)


TileContext._commit_instruction = _commit_split

# ------------------------------------------------------------- dimensions
NGRAMS = (2, 3, 4, 5)
P, S, T = 64, 30, 128
V, E, H, NF, U = 50000, 200, 256, 128, 20
NCORES = 8
DPC = P // NCORES          # docs per core
NSENT = DPC * S            # sentences per core
BLK = 4                    # sentences per conv block
NBLK = NSENT // BLK
E0, E1 = 128, E - 128      # embedding-dim chunks (contraction on PE)
PAIRS = [(k, j) for k in NGRAMS for j in range(k)]   # 14 (ngram, shift)
NPAIR = len(PAIRS)
G3 = 3 * H                 # 768 gate units
GMC = G3 // 128            # 6 gate-unit chunks
HMC = H // 128             # 2 hidden chunks
PB = H + U                 # 276
SELU_L = 1.0507009873554805
SELU_A = 1.6732632423543772

F32 = mybir.dt.float32
BF = mybir.dt.bfloat16
I32 = mybir.dt.int32
AX = mybir.AxisListType
ALU = mybir.AluOpType
ACT = mybir.ActivationFunctionType

_cache = {}


def _build_nc():
    nc = bass.Bass("TRN2", target_bir_lowering=False, debug=False,
                   num_devices=NCORES)

    # ---------------- dram I/O ----------------
    embed = nc.dram_tensor("embed", [V, E], BF, kind="ExternalInput")
    toks = nc.dram_tensor("toks", [128, NSENT], I32, kind="ExternalInput")
    convw_d = nc.dram_tensor("convw", [128, NPAIR * 2 * 128], BF,
                             kind="ExternalInput")
    convb_d = nc.dram_tensor("convb", [128, 4], F32, kind="ExternalInput")
    projw_d = nc.dram_tensor("projw", [128, 4 * 2 * 128], BF,
                             kind="ExternalInput")
    projb_d = nc.dram_tensor("projb", [128, 2], F32, kind="ExternalInput")
    ident_d = nc.dram_tensor("ident", [128, 128], BF, kind="ExternalInput")
    # sentence GRU (dir-major: fwd, bwd)
    swih_d = nc.dram_tensor("swih", [128, 2 * 2 * GMC * 128], BF,
                            kind="ExternalInput")
    swhh_d = nc.dram_tensor("swhh", [128, 2 * 2 * GMC * 128], BF,
                            kind="ExternalInput")
    sgxb_d = nc.dram_tensor("sgxb", [128, 2 * GMC], F32, kind="ExternalInput")
    # doc GRU
    rwih_d = nc.dram_tensor("rwih", [128, 2 * 3 * GMC * 128], BF,
                            kind="ExternalInput")
    rwhh_d = nc.dram_tensor("rwhh", [128, 2 * 2 * GMC * 128], BF,
                            kind="ExternalInput")
    rgxb_d = nc.dram_tensor("rgxb", [128, 2 * GMC], F32, kind="ExternalInput")
    # heads
    rfc1_d = nc.dram_tensor("rfc1", [128, 3 * 128], BF, kind="ExternalInput")
    rfcb1_d = nc.dram_tensor("rfcb1", [128, 1], F32, kind="ExternalInput")
    rfc2_d = nc.dram_tensor("rfc2", [128, 1], BF, kind="ExternalInput")
    pfc1_d = nc.dram_tensor("pfc1", [128, 2 * 128], BF, kind="ExternalInput")
    pfcb1_d = nc.dram_tensor("pfcb1", [128, 1], F32, kind="ExternalInput")
    pfc2_d = nc.dram_tensor("pfc2", [128, 1], BF, kind="ExternalInput")
    fcb2_d = nc.dram_tensor("fcb2", [1, 2], F32, kind="ExternalInput")
    ufbf_d = nc.dram_tensor("ufbf", [U, P], BF, kind="ExternalInput")

    out_d = nc.dram_tensor("out", [1, 1 + P], F32, kind="ExternalOutput")

    with TileContext(nc) as tc:
        with (
            tc.tile_pool(name="wts", bufs=1) as wts,
            tc.tile_pool(name="persist", bufs=1) as persist,
            tc.tile_pool(name="gather", bufs=4) as gpool,
            tc.tile_pool(name="xt", bufs=4) as xtpool,
            tc.tile_pool(name="step", bufs=6) as spool,
            # PSUM budget (8 banks): ps0+ps1 bufs=2 -> 4, mm bufs=2 -> 2,
            # ghp bufs=2 -> 2.  All matmul tiles <=1 bank.
            tc.tile_pool(name="pt", bufs=2, space="PSUM") as pt,
            tc.tile_pool(name="pc", bufs=2, space="PSUM") as pc,
            tc.tile_pool(name="pg", bufs=2, space="PSUM") as pg,
            tc.tile_pool(name="dram", bufs=1, space="DRAM") as dram,
        ):
            # ------------- load weights/constants to SBUF -------------
            def wtile(shape, dt, src, tag):
                t = wts.tile(shape, dt, tag=tag, name=tag)
                nc.sync.dma_start(t[:], src)
                return t

            convw = wtile([128, NPAIR * 2 * 128], BF, convw_d[:], "convw")
            convb = wtile([128, 4], F32, convb_d[:], "convb")
            projw = wtile([128, 4 * 2 * 128], BF, projw_d[:], "projw")
            projb = wtile([128, 2], F32, projb_d[:], "projb")
            ident = wtile([128, 128], BF, ident_d[:], "ident")
            swih = wtile([128, 2 * 2 * GMC * 128], BF, swih_d[:], "swih")
            swhh = wtile([128, 2 * 2 * GMC * 128], BF, swhh_d[:], "swhh")
            sgxb = wtile([128, 2 * GMC], F32, sgxb_d[:], "sgxb")
            rwih = wtile([128, 2 * 3 * GMC * 128], BF, rwih_d[:], "rwih")
            rwhh = wtile([128, 2 * 2 * GMC * 128], BF, rwhh_d[:], "rwhh")
            rgxb = wtile([128, 2 * GMC], F32, rgxb_d[:], "rgxb")
            rfc1 = wtile([128, 3 * 128], BF, rfc1_d[:], "rfc1")
            rfcb1 = wtile([128, 1], F32, rfcb1_d[:], "rfcb1")
            rfc2 = wtile([128, 1], BF, rfc2_d[:], "rfc2")
            pfc1 = wtile([128, 2 * 128], BF, pfc1_d[:], "pfc1")
            pfcb1 = wtile([128, 1], F32, pfcb1_d[:], "pfcb1")
            pfc2 = wtile([128, 1], BF, pfc2_d[:], "pfc2")
            fcb2 = wtile([1, 2], F32, fcb2_d[:], "fcb2")
            ufbf = wtile([U, P], BF, ufbf_d[:], "ufbf")
            toksb = wtile([128, NSENT], I32, toks[:], "toksb")

            def sw(dir_, kc, mc):   # sentence-GRU Wih^T block [128,128]
                return swih[:, ((dir_ * 2 + kc) * GMC + mc) * 128:
                            ((dir_ * 2 + kc) * GMC + mc) * 128 + 128]

            def sh(dir_, kc, mc):
                return swhh[:, ((dir_ * 2 + kc) * GMC + mc) * 128:
                            ((dir_ * 2 + kc) * GMC + mc) * 128 + 128]

            def rw(dir_, kc, mc):
                return rwih[:, ((dir_ * 3 + kc) * GMC + mc) * 128:
                            ((dir_ * 3 + kc) * GMC + mc) * 128 + 128]

            def rh(dir_, kc, mc):
                return rwhh[:, ((dir_ * 2 + kc) * GMC + mc) * 128:
                            ((dir_ * 2 + kc) * GMC + mc) * 128 + 128]

            # persistent activations
            pooled = [persist.tile([128, NSENT], BF, tag=f"pooled{i}",
                                   name=f"pooled{i}") for i in range(4)]
            svT = persist.tile([128, 2 * NSENT], BF, tag="svT")
            gxall = persist.tile([128, 2 * GMC * NSENT], F32, tag="gxall")
            hall32 = persist.tile([128, 2 * 2 * DPC], F32, tag="hall32")
            hallbf = persist.tile([128, 2 * 2 * DPC], BF, tag="hallbf")
            docv = persist.tile([128, 2 * DPC], F32, tag="docv")
            pbT = [persist.tile([128, P], F32, tag=f"pbT{c}", name=f"pbT{c}")
                   for c in range(2)]
            pbbf = [persist.tile([128, P], BF, tag=f"pbbf{c}", name=f"pbbf{c}")
                    for c in range(2)]
            gxrall = persist.tile([128, 2 * GMC * P], F32, tag="gxrall")
            hrall32 = persist.tile([128, 4], F32, tag="hrall32")
            hrallbf = persist.tile([128, 4], BF, tag="hrallbf")
            out_sb = persist.tile([1, 1 + P], F32, tag="out_sb")

            # ================= stage A: CNN encoder =================
            for b in range(NBLK):
                # 256-col slot per sentence: cols 200..256 are padding so the
                # E1 transpose below can read a full 128-wide window
                gbuf = gpool.tile([128, BLK * 256], BF, tag="gbuf")
                for s in range(BLK):
                    nc.gpsimd.indirect_dma_start(
                        out=gbuf[:, s * 256: s * 256 + E],
                        out_offset=None,
                        in_=embed[:],
                        in_offset=bass.IndirectOffsetOnAxis(
                            ap=toksb[:, b * BLK + s: b * BLK + s + 1], axis=0
                        ),
                    )
                # transpose to [E, tok] layout (PE transpose-mode)
                ps0 = pt.tile([128, BLK * 128], BF, tag="ps0")
                ps1 = pt.tile([128, BLK * 128], BF, tag="ps1")
                for s in range(BLK):
                    nc.tensor.transpose(
                        ps0[:, s * 128:(s + 1) * 128],
                        gbuf[:, s * 256: s * 256 + E0], ident[:])
                    nc.tensor.transpose(
                        ps1[:E1, s * 128:(s + 1) * 128],
                        gbuf[:, s * 256 + E0: s * 256 + E], ident[:])
                xt0 = xtpool.tile([128, BLK * 128], BF, tag="xt0")
                xt1 = xtpool.tile([128, BLK * 128], BF, tag="xt1")
                nc.scalar.copy(xt0[:], ps0[:])
                nc.vector.tensor_copy(xt1[:E1, :], ps1[:E1, :])

                xts = (xt0, xt1)
                kdim = (E0, E1)
                for ki, k in enumerate(NGRAMS):
                    L = T - k + 1
                    cps = pc.tile([128, BLK * L], F32, tag="mm")
                    n_mm = 2 * k
                    mi = 0
                    for j in range(k):
                        for c in range(2):
                            pi = PAIRS.index((k, j))
                            lhsT = convw[:kdim[c],
                                         (pi * 2 + c) * 128:(pi * 2 + c) * 128 + 128]
                            rhs = (xts[c][:kdim[c], :]
                                   .rearrange("p (s t) -> p s t", s=BLK)
                                   [:, :, j:j + L])
                            nc.tensor.matmul(
                                out=cps[:].rearrange("p (s t) -> p s t", s=BLK),
                                lhsT=lhsT, rhs=rhs,
                                start=(mi == 0), stop=(mi == n_mm - 1))
                            mi += 1
                    # max over time commutes with relu and the per-filter
                    # bias: pool first (cheap epilogue), relu+bias on [128,4]
                    poolf = spool.tile([128, BLK], F32, tag="poolf")
                    nc.vector.tensor_reduce(
                        out=poolf[:],
                        in_=cps[:].rearrange("p (s t) -> p s t", s=BLK),
                        axis=AX.X, op=ALU.max)
                    nc.scalar.activation(pooled[ki][:, b * BLK:(b + 1) * BLK],
                                         poolf[:], ACT.Relu,
                                         bias=convb[:, ki:ki + 1])

            # projection: svT[h] = proj_w.T @ pooled  (+bias)
            for h in range(2):
                pps = pc.tile([128, NSENT], F32, tag="mm")
                for c in range(4):
                    nc.tensor.matmul(
                        out=pps[:],
                        lhsT=projw[:, (c * 2 + h) * 128:(c * 2 + h) * 128 + 128],
                        rhs=pooled[c][:],
                        start=(c == 0), stop=(c == 3))
                nc.scalar.activation(svT[:, h * NSENT:(h + 1) * NSENT], pps[:],
                                     ACT.Identity, bias=projb[:, h:h + 1])

            # ================= stage B: sentence BiGRU =================
            # gx[d][:, mc*NSENT + s] = (Wih_d^T @ sv)[gate chunk mc, sentence s]
            for d in range(2):
                for mc in range(GMC):
                    gps = pc.tile([128, NSENT], F32, tag="mm")
                    for kc in range(2):
                        nc.tensor.matmul(
                            out=gps[:], lhsT=sw(d, kc, mc),
                            rhs=svT[:, kc * NSENT:(kc + 1) * NSENT],
                            start=(kc == 0), stop=(kc == 1))
                    nc.scalar.activation(
                        gxall[:, (d * GMC + mc) * NSENT:
                              (d * GMC + mc + 1) * NSENT], gps[:],
                        ACT.Identity, bias=sgxb[:, d * GMC + mc: d * GMC + mc + 1])
            nc.vector.memset(hall32[:], 0.0)
            nc.vector.memset(hallbf[:], 0.0)

            # both directions processed per step in the same instructions;
            # custom APs pair dir-0 column t with dir-1 column S-1-t.
            gxt = gxall[:].ap[0]
            for t in range(S):
                ghp = pg.tile([128, 2 * GMC * DPC], F32, tag="ghp")
                for d in range(2):
                    for mc in range(GMC):
                        for kc in range(2):
                            nc.tensor.matmul(
                                out=ghp[:, (d * GMC + mc) * DPC:
                                        (d * GMC + mc + 1) * DPC],
                                lhsT=sh(d, kc, mc),
                                rhs=hallbf[:, (d * 2 + kc) * DPC:
                                           (d * 2 + kc + 1) * DPC],
                                start=(kc == 0), stop=(kc == 1))
                dstep = GMC * NSENT + (S - 1 - 2 * t) * 1

                def gxsl(g0, ng):
                    return bass.AP(gxall[:].tensor, gxall[:].offset
                                   + g0 * NSENT + t,
                                   [gxt, [dstep, 2], [NSENT, ng], [S, DPC]])

                def ghsl(g0, ng):
                    return (ghp[:].rearrange("p (d g s) -> p d g s", d=2, g=GMC)
                            [:, :, g0:g0 + ng, :])
                rz = spool.tile([128, 2 * 4 * DPC], F32, tag="rz")
                nc.vector.tensor_add(
                    rz[:].rearrange("p (d g s) -> p d g s", d=2, g=4),
                    gxsl(0, 4), ghsl(0, 4))
                nc.scalar.activation(rz[:], rz[:], ACT.Sigmoid)
                npre = spool.tile([128, 2 * 2 * DPC], F32, tag="npre")
                nc.vector.tensor_mul(
                    npre[:].rearrange("p (d g s) -> p d g s", d=2, g=2),
                    rz[:].rearrange("p (d g s) -> p d g s", d=2, g=4)[:, :, 0:2, :],
                    ghsl(4, 2))
                nc.vector.tensor_add(
                    npre[:].rearrange("p (d g s) -> p d g s", d=2, g=2),
                    npre[:].rearrange("p (d g s) -> p d g s", d=2, g=2),
                    gxsl(4, 2))
                nc.scalar.activation(npre[:], npre[:], ACT.Tanh)
                hmn = spool.tile([128, 2 * 2 * DPC], F32, tag="hmn")
                nc.vector.tensor_sub(hmn[:], hall32[:], npre[:])
                nc.vector.tensor_mul(
                    hmn[:].rearrange("p (d g s) -> p d g s", d=2, g=2),
                    rz[:].rearrange("p (d g s) -> p d g s", d=2, g=4)[:, :, 2:4, :],
                    hmn[:].rearrange("p (d g s) -> p d g s", d=2, g=2))
                nc.vector.tensor_add(hall32[:], npre[:], hmn[:])
                nc.vector.tensor_add(hallbf[:], npre[:], hmn[:])

            nc.vector.tensor_add(docv[:], hall32[:, :2 * DPC],
                                 hall32[:, 2 * DPC:4 * DPC])

            # ================= AllGather doc vectors =================
            bounce = dram.tile([2, 128, DPC], F32, tag="bounce")
            gath = dram.tile([NCORES, 2, 128, DPC], F32, tag="gath")
            for c in range(2):
                nc.sync.dma_start(bounce[c], docv[:, c * DPC:(c + 1) * DPC])
            nc.gpsimd.collective_compute(
                "AllGather", ALU.bypass,
                replica_groups=[list(range(NCORES))],
                ins=[bounce.opt()], outs=[gath.opt()])
            for c in range(2):
                nc.sync.dma_start(
                    pbT[c][:],
                    gath[:, c, :, :].rearrange("a p b -> p a b"))
                nc.vector.tensor_copy(pbbf[c][:], pbT[c][:])

            # ================= r_stars head =================
            rps = pg.tile([128, P], F32, tag="ghp")
            for c in range(2):
                nc.tensor.matmul(out=rps[:], lhsT=rfc1[:, c * 128:(c + 1) * 128],
                                 rhs=pbbf[c][:], start=(c == 0), stop=False)
            nc.tensor.matmul(out=rps[:], lhsT=rfc1[:U, 2 * 128:3 * 128],
                             rhs=ufbf[:], start=False, stop=True)
            selu_in = spool.tile([128, P], F32, tag="selu_in")
            nc.scalar.activation(selu_in[:], rps[:], ACT.Identity,
                                 bias=rfcb1[:])

            def selu_apply(dst_bf, x, cols):
                # dst = SELU_L*relu(x) + SELU_L*SELU_A*(exp(min(x,0))-1)
                pos = spool.tile([128, P], F32, tag="selu_pos")
                neg = spool.tile([128, P], F32, tag="selu_neg")
                nc.scalar.activation(pos[:, :cols], x, ACT.Relu, scale=SELU_L)
                nc.vector.tensor_scalar_min(neg[:, :cols], x, 0.0)
                nc.scalar.activation(neg[:, :cols], neg[:, :cols], ACT.Exp)
                nc.vector.tensor_scalar(
                    out=neg[:, :cols], in0=neg[:, :cols],
                    scalar1=SELU_L * SELU_A, scalar2=-SELU_L * SELU_A,
                    op0=ALU.mult, op1=ALU.add)
                nc.vector.tensor_add(neg[:, :cols], neg[:, :cols], pos[:, :cols])
                nc.vector.tensor_copy(dst_bf, neg[:, :cols])

            selu_bf = spool.tile([128, P], BF, tag="selu_bf")
            selu_apply(selu_bf[:], selu_in[:], P)
            rsp = pg.tile([1, P], F32, tag="ghp")
            nc.tensor.matmul(out=rsp[:], lhsT=rfc2[:], rhs=selu_bf[:],
                             start=True, stop=True)
            nc.scalar.activation(out_sb[:, 1:1 + P], rsp[:], ACT.Identity,
                                 bias=fcb2[:, 0:1])

            # ================= doc-sequence BiGRU =================
            for d in range(2):
                for mc in range(GMC):
                    gps = pc.tile([128, P], F32, tag="mm")
                    nc.tensor.matmul(out=gps[:], lhsT=rw(d, 0, mc),
                                     rhs=pbbf[0][:], start=True, stop=False)
                    nc.tensor.matmul(out=gps[:], lhsT=rw(d, 1, mc),
                                     rhs=pbbf[1][:], start=False, stop=False)
                    nc.tensor.matmul(out=gps[:], lhsT=rw(d, 2, mc)[:U, :],
                                     rhs=ufbf[:], start=False, stop=True)
                    nc.scalar.activation(
                        gxrall[:, (d * GMC + mc) * P:(d * GMC + mc + 1) * P],
                        gps[:], ACT.Identity,
                        bias=rgxb[:, d * GMC + mc: d * GMC + mc + 1])
            nc.vector.memset(hrall32[:], 0.0)
            nc.vector.memset(hrallbf[:], 0.0)

            gxrt = gxrall[:].ap[0]
            for t in range(P):
                ghp = pg.tile([128, 2 * GMC], F32, tag="ghp")
                for d in range(2):
                    for mc in range(GMC):
                        for kc in range(2):
                            nc.tensor.matmul(
                                out=ghp[:, d * GMC + mc: d * GMC + mc + 1],
                                lhsT=rh(d, kc, mc),
                                rhs=hrallbf[:, d * 2 + kc: d * 2 + kc + 1],
                                start=(kc == 0), stop=(kc == 1))
                dstep = GMC * P + (P - 1 - 2 * t) * 1

                def gxrsl(g0, ng):
                    return bass.AP(gxrall[:].tensor, gxrall[:].offset
                                   + g0 * P + t,
                                   [gxrt, [dstep, 2], [P, ng]])

                def ghrsl(g0, ng):
                    return (ghp[:].rearrange("p (d g) -> p d g", d=2)
                            [:, :, g0:g0 + ng])
                rz = spool.tile([128, 8], F32, tag="rzr")
                nc.vector.tensor_add(
                    rz[:].rearrange("p (d g) -> p d g", d=2),
                    gxrsl(0, 4), ghrsl(0, 4))
                nc.scalar.activation(rz[:], rz[:], ACT.Sigmoid)
                npre = spool.tile([128, 4], F32, tag="nprer")
                nc.vector.tensor_mul(
                    npre[:].rearrange("p (d g) -> p d g", d=2),
                    rz[:].rearrange("p (d g) -> p d g", d=2)[:, :, 0:2],
                    ghrsl(4, 2))
                nc.vector.tensor_add(
                    npre[:].rearrange("p (d g) -> p d g", d=2),
                    npre[:].rearrange("p (d g) -> p d g", d=2),
                    gxrsl(4, 2))
                nc.scalar.activation(npre[:], npre[:], ACT.Tanh)
                hmn = spool.tile([128, 4], F32, tag="hmnr")
                nc.vector.tensor_sub(hmn[:], hrall32[:], npre[:])
                nc.vector.tensor_mul(
                    hmn[:].rearrange("p (d g) -> p d g", d=2),
                    rz[:].rearrange("p (d g) -> p d g", d=2)[:, :, 2:4],
                    hmn[:].rearrange("p (d g) -> p d g", d=2))
                nc.vector.tensor_add(hrall32[:], npre[:], hmn[:])
                nc.vector.tensor_add(hrallbf[:], npre[:], hmn[:])

            # ================= p_stars head =================
            hn = spool.tile([128, 2], F32, tag="hn")
            hnbf = spool.tile([128, 2], BF, tag="hnbf")
            nc.vector.tensor_add(hn[:], hrall32[:, 0:2], hrall32[:, 2:4])
            nc.vector.tensor_copy(hnbf[:], hn[:])
            pps1 = pg.tile([128, 1], F32, tag="ghp")
            for c in range(2):
                nc.tensor.matmul(out=pps1[:], lhsT=pfc1[:, c * 128:(c + 1) * 128],
                                 rhs=hnbf[:, c:c + 1],
                                 start=(c == 0), stop=(c == 1))
            psel_in = spool.tile([128, 1], F32, tag="psel_in")
            nc.scalar.activation(psel_in[:], pps1[:], ACT.Identity, bias=pfcb1[:])
            psel_bf = spool.tile([128, 1], BF, tag="psel_bf")
            selu_apply(psel_bf[:], psel_in[:], 1)
            pstar = pg.tile([1, 1], F32, tag="ghp")
            nc.tensor.matmul(out=pstar[:], lhsT=pfc2[:], rhs=psel_bf[:],
                             start=True, stop=True)
            nc.scalar.activation(out_sb[:, 0:1], pstar[:], ACT.Identity,
                                 bias=fcb2[:, 1:2])

            nc.sync.dma_start(out_d[:], out_sb[:])

    return nc


def _prep_inputs(inputs, user_feats, params):
    """Host-side packing of per-core input maps."""
    embed_bf = np.asarray(params["embed"], np.float32).astype(BF16)

    convw = np.zeros((128, NPAIR * 2 * 128), BF16)
    for pi, (k, j) in enumerate(PAIRS):
        Wt = np.asarray(params["conv_w"][str(k)], np.float32)[:, :, j].T  # [E,NF]
        convw[:, (pi * 2 + 0) * 128:(pi * 2 + 1) * 128] = Wt[:E0].astype(BF16)
        convw[:E1, (pi * 2 + 1) * 128:(pi * 2 + 2) * 128] = Wt[E0:].astype(BF16)
    convb = np.zeros((128, 4), np.float32)
    for ki, k in enumerate(NGRAMS):
        convb[:, ki] = np.asarray(params["conv_b"][str(k)], np.float32)

    pw = np.asarray(params["proj_w"], np.float32)  # [512, 256]
    projw = np.zeros((128, 4 * 2 * 128), BF16)
    for c in range(4):
        for h in range(2):
            projw[:, (c * 2 + h) * 128:(c * 2 + h + 1) * 128] = (
                pw[c * 128:(c + 1) * 128, h * 128:(h + 1) * 128].astype(BF16))
    projb = np.asarray(params["proj_b"], np.float32).reshape(2, 128).T.copy()
    ident = np.eye(128, dtype=np.float32).astype(BF16)

    def pack_gru(p, din, nkc):
        wih = np.zeros((128, 2 * nkc * GMC * 128), BF16)
        whh = np.zeros((128, 2 * 2 * GMC * 128), BF16)
        gxb = np.zeros((128, 2 * GMC), np.float32)
        for di, dname in enumerate(("fwd", "bwd")):
            g = p[dname]
            wihT = np.zeros((nkc * 128, G3), np.float32)
            wihT[:din] = np.asarray(g["wih"], np.float32).T
            whhT = np.asarray(g["whh"], np.float32).T  # [256, 768]
            for kc in range(nkc):
                for mc in range(GMC):
                    wih[:, ((di * nkc + kc) * GMC + mc) * 128:
                        ((di * nkc + kc) * GMC + mc + 1) * 128] = (
                        wihT[kc * 128:(kc + 1) * 128,
                             mc * 128:(mc + 1) * 128].astype(BF16))
            for kc in range(2):
                for mc in range(GMC):
                    whh[:, ((di * 2 + kc) * GMC + mc) * 128:
                        ((di * 2 + kc) * GMC + mc + 1) * 128] = (
                        whhT[kc * 128:(kc + 1) * 128,
                             mc * 128:(mc + 1) * 128].astype(BF16))
            bih = np.asarray(g["bih"], np.float32)
            bhh = np.asarray(g["bhh"], np.float32)
            assert np.all(bhh[2 * H:] == 0.0), "nonzero b_hn unsupported"
            btot = bih.copy()
            btot[:2 * H] += bhh[:2 * H]
            gxb[:, di * GMC:(di + 1) * GMC] = btot.reshape(GMC, 128).T
        return wih, whh, gxb

    swih, swhh, sgxb = pack_gru(params["s_gru"], H, 2)
    rwih, rwhh, rgxb = pack_gru(params["r_gru"], PB, 3)

    rf1 = np.asarray(params["rfc_w1"], np.float32)  # [276, 128]
    rfc1 = np.zeros((128, 3 * 128), BF16)
    rfc1[:, 0:128] = rf1[:128].astype(BF16)
    rfc1[:, 128:256] = rf1[128:256].astype(BF16)
    rfc1[:U, 256:384] = rf1[256:].astype(BF16)
    rfcb1 = np.asarray(params["rfc_b1"], np.float32).reshape(128, 1)
    rfc2 = np.asarray(params["rfc_w2"], np.float32).reshape(128, 1).astype(BF16)
    pf1 = np.asarray(params["pfc_w1"], np.float32)  # [256, 128]
    pfc1 = np.zeros((128, 2 * 128), BF16)
    pfc1[:, 0:128] = pf1[:128].astype(BF16)
    pfc1[:, 128:256] = pf1[128:].astype(BF16)
    pfcb1 = np.asarray(params["pfc_b1"], np.float32).reshape(128, 1)
    pfc2 = np.asarray(params["pfc_w2"], np.float32).reshape(128, 1).astype(BF16)
    fcb2 = np.array([[np.asarray(params["rfc_b2"], np.float32).reshape(()),
                      np.asarray(params["pfc_b2"], np.float32).reshape(())]],
                    np.float32)

    uf = np.asarray(user_feats, np.float32)
    ufn = uf / np.maximum(np.linalg.norm(uf, axis=-1, keepdims=True), 1e-12)
    ufbf = ufn.T.astype(BF16).copy()  # [20, 64]

    toks_all = np.asarray(inputs, np.int32)  # [P, S, T]
    shared = dict(
        embed=embed_bf, convw=convw, convb=convb, projw=projw, projb=projb,
        ident=ident, swih=swih, swhh=swhh, sgxb=sgxb, rwih=rwih, rwhh=rwhh,
        rgxb=rgxb, rfc1=rfc1, rfcb1=rfcb1, rfc2=rfc2, pfc1=pfc1, pfcb1=pfcb1,
        pfc2=pfc2, fcb2=fcb2, ufbf=ufbf,
    )
    in_maps = []
    for c in range(NCORES):
        t = toks_all[c * DPC:(c + 1) * DPC]          # [8, 30, 128]
        toks = np.ascontiguousarray(
            t.reshape(NSENT, T).T.astype(np.int32))  # [128, 240]
        in_maps.append(dict(shared, toks=toks))
    return in_maps


def kernel(inputs, sent_lengths, sent_counts, user_feats, params):
    if "nc" not in _cache:
        _cache["nc"] = _build_nc()
    nc = _cache["nc"]
    in_maps = _prep_inputs(inputs, user_feats, params)
    res = run_bass_kernel_spmd(nc, in_maps, list(range(NCORES)))
    out = res.results[0]["out"]
    p_stars = np.float32(out[0, 0])
    r_stars = out[0, 1:1 + P].astype(np.float32)
    return np.asarray(p_stars, np.float32), r_stars


def run_traced(inputs, sent_lengths, sent_counts, user_feats, params):
    """Like kernel() but returns (outputs, BassKernelResults-with-trace)."""
    if "nc" not in _cache:
        _cache["nc"] = _build_nc()
    nc = _cache["nc"]
    in_maps = _prep_inputs(inputs, user_feats, params)
    res = run_bass_kernel_spmd(nc, in_maps, list(range(NCORES)), trace=True)
    out = res.results[0]["out"]
    p_stars = np.float32(out[0, 0])
    r_stars = out[0, 1:1 + P].astype(np.float32)
    return (np.asarray(p_stars, np.float32), r_stars), res
